# revision 15
# baseline (speedup 1.0000x reference)
"""MoE (threshold top-k routing, eval capacity) Trainium2 Bass kernel.

Strategy: data-parallel over the batch dim b (B=8 -> one batch element per
NeuronCore), full expert set computed locally on each core (no collectives).

Per-core program (N=2048 tokens, D=512, E=16, HID=2048, CAP=256):
  1. Gating logits via fp32 PE matmul (precision-critical: routing threshold
     margins are ~3e-5, so the gate path stays fp32 end-to-end).
  2. Softmax on ScalarE/VectorE; threshold top-k WITHOUT sorting via pairwise
     prob comparisons (expert e selected iff sum of probs strictly greater --
     ties broken by index -- is < 0.8).  Bit-matches jnp argsort semantics.
  3. Capacity: exclusive cumsum over tokens via the DVE scan instruction on an
     expert-major [16, 2048] layout; summed-slot quirk (pos_tok = sum over
     experts) reproduced exactly.
  4. Dispatch/combine as one-hot matmuls (handles duplicate-slot collisions by
     summation, exactly like the reference einsum).
  5. Expert FFN (gelu) with bf16 weights x float32r activations, fp32 PSUM.
"""

import sys

import numpy as np

sys.path.insert(0, "/opt/trn_rl_repo")

import ml_dtypes  # noqa: E402

import concourse.bass as bass  # noqa: E402
import concourse.mybir as mybir  # noqa: E402
import concourse.tile as tile  # noqa: E402
from concourse import bacc  # noqa: E402
from concourse.bass import ds, ts  # noqa: E402
from concourse.masks import make_identity  # noqa: E402

F32 = mybir.dt.float32
F32R = mybir.dt.float32r
BF16 = mybir.dt.bfloat16
I32 = mybir.dt.int32
AX = mybir.AxisListType
OP = mybir.AluOpType
AF = mybir.ActivationFunctionType

P = 128
B, N, D, E, HID, CAP = 8, 2048, 512, 16, 2048, 256
TC, DC, HC, CC = N // P, D // P, HID // P, CAP // P
NHALF = N // 2
THRESH = 0.8
LOSS_COEF = 0.01
NCORES = 8

BF16_NP = ml_dtypes.bfloat16


def _r(ap):
    """View an fp32 AP as float32r for 1-cycle/row PE matmuls."""
    return ap.bitcast(F32R)


def build_program():
    nc = bacc.Bacc(
        "TRN2",
        target_bir_lowering=False,
        debug=False,
        enable_asserts=False,
        num_devices=1,
    )

    x_ap = nc.dram_tensor("x", [N, D], F32, kind="ExternalInput").ap()
    wg_ap = nc.dram_tensor("wg", [D, E], F32, kind="ExternalInput").ap()
    # w1s[e, hc, p, dc, hp] = w1[e, dc*128+p, hc*128+hp]  (host-prepped)
    w1_ap = nc.dram_tensor("w1s", [E, HC, P, DC, P], BF16, kind="ExternalInput").ap()
    w2_ap = nc.dram_tensor("w2b", [E, HID, D], BF16, kind="ExternalInput").ap()
    out_ap = nc.dram_tensor("out", [N, D], F32, kind="ExternalOutput").ap()
    lossp_ap = nc.dram_tensor("lossp", [1, 1], F32, kind="ExternalOutput").ap()
    # internal DRAM bounce buffers
    eod_ap = nc.dram_tensor("eod", [E, CC, P, D], F32R).ap()
    wmd_ap = nc.dram_tensor("wmd", [E, N], F32).ap()

    out_t = out_ap.rearrange("(t p) d -> t p d", p=P)

    with tile.TileContext(nc) as tcx:
        _emit(tcx, x_ap, wg_ap, w1_ap, w2_ap, out_t, lossp_ap, eod_ap, wmd_ap)

    nc.compile()
    return nc


def _emit(tcx, x_ap, wg_ap, w1_ap, w2_ap, out_t, lossp_ap, eod_ap, wmd_ap):
    from contextlib import ExitStack

    nc = tcx.nc
    ctx = ExitStack()
    const = ctx.enter_context(tcx.tile_pool(name="const", bufs=1))
    keep = ctx.enter_context(tcx.tile_pool(name="keep", bufs=1))
    ctxR = ExitStack()
    rp = ctxR.enter_context(tcx.tile_pool(name="routing", bufs=1))
    pw = ctxR.enter_context(tcx.tile_pool(name="pairwise", bufs=2))
    psT = ctxR.enter_context(tcx.tile_pool(name="psT", bufs=6, space="PSUM"))

    # ---------------- constants ----------------
    ident = const.tile([P, P], F32, name="ident")
    make_identity(nc, ident)
    ones16 = const.tile([E, 1], F32, name="ones16")
    nc.vector.memset(ones16, 1.0)
    iota_ci = const.tile([P, CAP], I32, name="iota_ci")
    nc.gpsimd.iota(iota_ci, pattern=[[1, CAP]], base=0, channel_multiplier=0)
    iota_c = const.tile([P, CAP], F32, name="iota_c")
    nc.vector.tensor_copy(iota_c, iota_ci)
    iota_pi = const.tile([P, CC], I32, name="iota_pi")
    nc.gpsimd.iota(iota_pi, pattern=[[P, CC]], base=0, channel_multiplier=1)
    iota_p = const.tile([P, CC], F32, name="iota_p")
    nc.vector.tensor_copy(iota_p, iota_pi)
    # tri[p, ep, e] = 1.0 where e > ep else 0.0 (tie-break mask)
    tri = const.tile([P, E, E], F32, name="tri")
    nc.gpsimd.memset(tri, 1.0)
    nc.gpsimd.affine_select(
        out=tri, in_=tri, pattern=[[-1, E], [1, E]], base=0,
        channel_multiplier=0, compare_op=OP.is_gt, fill=0.0,
    )

    # ---------------- load x, wg ----------------
    x_sb = rp.tile([P, TC, D], F32, name="x_sb")
    nc.sync.dma_start(x_sb, x_ap.rearrange("(t p) d -> p t d", p=P))
    wg_sb = rp.tile([P, DC, E], F32, name="wg_sb")
    nc.sync.dma_start(wg_sb, wg_ap.rearrange("(c p) e -> p c e", p=P))

    # ---------------- transpose x -> xT [d-part, dc, t] ----------------
    with tcx.tile_pool(name="xt", bufs=1) as xtp:
        xT = xtp.tile([P, DC, N], F32, name="xT")
        for t in range(TC):
            for dc in range(DC):
                ps = psT.tile([P, P], F32, tag="ps")
                nc.tensor.transpose(ps, x_sb[:, t, ds(dc * P, P)], ident)
                nc.vector.tensor_copy(xT[:, dc, ds(t * P, P)], ps)

        # ---------------- gating logits [t-part, tc, e] (full fp32) ------
        lg = rp.tile([P, TC, E], F32, name="lg")
        for t in range(TC):
            ps = psT.tile([P, E], F32, tag="ps")
            for dc in range(DC):
                nc.tensor.matmul(
                    ps, lhsT=xT[:, dc, ts(t, P)], rhs=wg_sb[:, dc, :],
                    start=(dc == 0), stop=(dc == DC - 1),
                )
            nc.vector.tensor_copy(lg[:, t, :], ps)

    # persistent f32r copy of x for the dispatch matmuls (after xT freed)
    ctxX = ExitStack()
    xrp = ctxX.enter_context(tcx.tile_pool(name="xrp", bufs=1, side="right"))
    x_r = xrp.tile([P, TC, D], F32R, name="x_r")
    nc.vector.tensor_copy(x_r, x_sb)

    # ---------------- softmax (fp32) ----------------
    rmax = rp.tile([P, TC], F32, name="rmax")
    nc.vector.reduce_max(rmax, lg, axis=AX.X)
    sh = rp.tile([P, TC, E], F32, name="sh")
    nc.vector.tensor_sub(sh, lg, rmax[:, :, None].to_broadcast([P, TC, E]))
    u = rp.tile([P, TC, E], F32, name="u")
    nc.scalar.activation(u, sh, AF.Exp)
    usum = rp.tile([P, TC], F32, name="usum")
    nc.vector.reduce_sum(usum, u, axis=AX.X)
    rin = rp.tile([P, TC], F32, name="rin")
    nc.vector.reciprocal(rin, usum)
    p_sb = rp.tile([P, TC, E], F32, name="p_sb")
    nc.vector.tensor_mul(p_sb, u, rin[:, :, None].to_broadcast([P, TC, E]))

    # ------------- threshold top-k via pairwise comparisons -------------
    # S[t, e] = sum_{e'} p[t,e'] * [ p_e' > p_e  or (p_e' == p_e and e' < e) ]
    # mask[t, e] = S[t, e] < THRESH      (== reference argsort/cumsum mask)
    S = rp.tile([P, TC, E], F32, name="S")
    nc.vector.memset(S, 0.0)
    for ep in range(E):
        pb = p_sb[:, :, ep : ep + 1].to_broadcast([P, TC, E])
        gt = pw.tile([P, TC, E], F32, tag="gt")
        nc.vector.tensor_tensor(gt, pb, p_sb, OP.is_gt)
        eq = pw.tile([P, TC, E], F32, tag="eq")
        nc.vector.tensor_tensor(eq, pb, p_sb, OP.is_equal)
        # ties only count for columns e > ep
        m = pw.tile([P, TC, E], F32, tag="m")
        nc.vector.tensor_mul(m, eq, tri[:, ep : ep + 1, :].to_broadcast([P, TC, E]))
        nc.vector.tensor_add(m, m, gt)
        t2 = pw.tile([P, TC, E], F32, tag="t2")
        nc.vector.tensor_mul(t2, m, pb)
        nc.vector.tensor_add(S, S, t2)

    mask = rp.tile([P, TC, E], F32, name="mask")
    nc.vector.tensor_single_scalar(mask, S, THRESH, OP.is_lt)
    selp = rp.tile([P, TC, E], F32, name="selp")
    nc.vector.tensor_mul(selp, p_sb, mask)
    wsum = rp.tile([P, TC], F32, name="wsum")
    nc.vector.reduce_sum(wsum, selp, axis=AX.X)
    winv = rp.tile([P, TC], F32, name="winv")
    nc.vector.reciprocal(winv, wsum)
    wts = rp.tile([P, TC, E], F32, name="wts")
    nc.vector.tensor_mul(wts, selp, winv[:, :, None].to_broadcast([P, TC, E]))

    # ------------- transpose mask/wts/p to expert-major [E, N] -------------
    mask_T = rp.tile([E, N], F32, name="mask_T")
    wts_T = rp.tile([E, N], F32, name="wts_T")
    p_T = rp.tile([E, N], F32, name="p_T")
    for t in range(TC):
        for src, dst in ((mask, mask_T), (wts, wts_T), (p_sb, p_T)):
            ps = psT.tile([E, P], F32, tag="ps")
            nc.tensor.transpose(ps, src[:, t, :], ident)
            nc.vector.tensor_copy(dst[:, ds(t * P, P)], ps)

    # ------------- capacity (exclusive cumsum over tokens) -------------
    cum = rp.tile([E, N], F32, name="cum")
    nc.vector.tensor_tensor_scan(cum, mask_T, mask_T, 0.0, OP.add, OP.bypass)
    pos = rp.tile([E, N], F32, name="pos")
    nc.vector.tensor_sub(pos, cum, mask_T)
    mask2_T = rp.tile([E, N], F32, name="mask2_T")
    nc.vector.scalar_tensor_tensor(
        mask2_T, in0=pos, scalar=float(CAP), in1=mask_T, op0=OP.is_lt, op1=OP.mult
    )
    pos2 = rp.tile([E, N], F32, name="pos2")
    nc.vector.tensor_mul(pos2, pos, mask2_T)
    wm_T = rp.tile([E, N], F32, name="wm_T")
    nc.vector.tensor_mul(wm_T, wts_T, mask2_T)
    nc.sync.dma_start(wmd_ap, wm_T)

    # ------------- aux loss partial: sum_e mean_t(p) * mean_t(mask2) -----
    proxs = rp.tile([E, 1], F32, name="proxs")
    nc.vector.reduce_sum(proxs, p_T, axis=AX.X)
    denss = rp.tile([E, 1], F32, name="denss")
    nc.vector.reduce_sum(denss, mask2_T, axis=AX.X)
    prod = rp.tile([E, 1], F32, name="prod")
    nc.vector.tensor_mul(prod, proxs, denss)
    psl = psT.tile([1, 1], F32, tag="ps")
    nc.tensor.matmul(psl, lhsT=prod, rhs=ones16, start=True, stop=True)
    lp_sb = rp.tile([1, 1], F32, name="lp_sb")
    nc.scalar.copy(lp_sb, psl)
    nc.sync.dma_start(lossp_ap, lp_sb)

    # ------------- summed slot index s(t) = sum_e pos2  -------------
    # expert-major broadcast copy: s_bcast[c-part, t]
    s_row = rp.tile([1, N], F32, name="s_row")
    for q in range(N // 512):
        ps = psT.tile([1, 512], F32, tag="ps")
        nc.tensor.matmul(
            ps, lhsT=ones16, rhs=pos2[:, ds(q * 512, 512)], start=True, stop=True
        )
        nc.vector.tensor_copy(s_row[:, ds(q * 512, 512)], ps)
    ones1 = const.tile([1, P], F32, name="ones1")
    nc.vector.memset(ones1, 1.0)
    s_bcast = rp.tile([P, N], F32, name="s_bcast")
    for q in range(N // 512):
        ps = psT.tile([P, 512], F32, tag="ps")
        nc.tensor.matmul(
            ps, lhsT=ones1, rhs=s_row[:, ds(q * 512, 512)], start=True, stop=True
        )
        nc.vector.tensor_copy(s_bcast[:, ds(q * 512, 512)], ps)

    # token-major mask2 and s: transpose back
    mask2_tok = keep.tile([P, TC, E], F32, name="mask2_tok")
    pos2_tok = rp.tile([P, TC, E], F32, name="pos2_tok")
    for t in range(TC):
        for src, dst in ((mask2_T, mask2_tok), (pos2, pos2_tok)):
            ps = psT.tile([P, E], F32, tag="ps")
            nc.tensor.transpose(ps, src[:, ds(t * P, P)], ident[:E, :E])
            nc.vector.tensor_copy(dst[:, t, :], ps)
    s_tok = rp.tile([P, TC], F32, name="s_tok")
    nc.vector.reduce_sum(s_tok, pos2_tok, axis=AX.X)

    # ------------- one-hot slot structures -------------
    oh_tok = keep.tile([P, TC, CAP], F32, name="oh_tok")  # [t, c] = (s(t)==c)
    nc.vector.tensor_tensor(
        oh_tok,
        s_tok[:, :, None].to_broadcast([P, TC, CAP]),
        iota_c[:, None, :].to_broadcast([P, TC, CAP]),
        OP.is_equal,
    )
    oh_T = keep.tile([P, CC, N], BF16, name="oh_T")  # [c, t] = (s(t)==c)
    for cc in range(CC):
        nc.vector.tensor_tensor(
            oh_T[:, cc, :],
            s_bcast,
            iota_p[:, cc : cc + 1].to_broadcast([P, N]),
            OP.is_equal,
        )

    ctxR.close()

    # ================= Phase A: dispatch + expert FFN =================
    with (
        tcx.tile_pool(name="mpool", bufs=2) as mpool,
        tcx.tile_pool(name="w1pool", bufs=3) as w1pool,
        tcx.tile_pool(name="w2pool", bufs=3) as w2pool,
        tcx.tile_pool(name="eipool", bufs=2) as eipool,
        tcx.tile_pool(name="hpool", bufs=2) as hpool,
        tcx.tile_pool(name="eopool", bufs=3) as eopool,
        tcx.tile_pool(name="psA", bufs=1, space="PSUM") as psA,
        tcx.tile_pool(name="psH", bufs=2, space="PSUM") as psH,
    ):
        for e in range(E):
            # M_e[t, c] = oh[t, c] * mask2[t, e]   (bf16, exact 0/1 weights)
            M_e = mpool.tile([P, TC, CAP], F32R, tag="M")
            for t in range(TC):
                nc.vector.tensor_scalar_mul(
                    M_e[:, t, :], oh_tok[:, t, :], mask2_tok[:, t, e : e + 1]
                )
            # EI_T[d, c] = sum_t x[t, d] * M_e[t, c]
            ei = eipool.tile([P, DC, CAP], BF16, tag="ei")
            for dc in range(DC):
                pse = psA.tile([P, CAP], F32, tag=f"ei{dc}")
                for t in range(TC):
                    nc.tensor.matmul(
                        pse,
                        lhsT=x_r[:, t, ds(dc * P, P)],
                        rhs=M_e[:, t, :],
                        start=(t == 0),
                        stop=(t == TC - 1),
                    )
                nc.scalar.copy(ei[:, dc, :], pse)
            # h[hid, c] = gelu(sum_d w1[d, hid] * EI_T[d, c])
            h = hpool.tile([P, HC, CAP], BF16, tag="h")
            for hc in range(HC):
                w1t = w1pool.tile([P, DC, P], BF16, tag="w1")
                nc.sync.dma_start(w1t, w1_ap[e, hc])
                psh = psH.tile([P, CAP], F32, tag="h")
                for dc in range(DC):
                    nc.tensor.matmul(
                        psh,
                        lhsT=w1t[:, dc, :],
                        rhs=ei[:, dc, :],
                        start=(dc == 0),
                        stop=(dc == DC - 1),
                    )
                nc.scalar.activation(h[:, hc, :], psh, AF.Gelu)
            # EO[c, d] = sum_hid h[hid, c] * w2[hid, d]
            pso = [
                psA.tile([P, D], F32, tag=f"eo{cc}", name=f"pso{cc}")
                for cc in range(CC)
            ]
            for hc in range(HC):
                w2t = w2pool.tile([P, D], BF16, tag="w2")
                nc.sync.dma_start(w2t, w2_ap[e, ds(hc * P, P), :])
                for cc in range(CC):
                    nc.tensor.matmul(
                        pso[cc],
                        lhsT=h[:, hc, ds(cc * P, P)],
                        rhs=w2t,
                        start=(hc == 0),
                        stop=(hc == HC - 1),
                    )
            for cc in range(CC):
                eo_sb = eopool.tile([P, D], F32R, tag="eo")
                nc.scalar.copy(eo_sb, pso[cc])
                nc.sync.dma_start(eod_ap[e, cc], eo_sb)

    ctxX.close()

    # ================= Phase B: combine =================
    with (
        tcx.tile_pool(name="cpool", bufs=2) as cpool,
        tcx.tile_pool(name="wmpool", bufs=2) as wmpool,
        tcx.tile_pool(name="eold", bufs=2) as eold,
        tcx.tile_pool(name="ostage", bufs=3) as ostage,
        tcx.tile_pool(name="psB", bufs=1, space="PSUM") as psB,
    ):
        for half in range(2):
            psO = [
                psB.tile([P, D], F32, tag=f"o{i}", name=f"psO{i}")
                for i in range(TC // 2)
            ]
            for e in range(E):
                wmb = wmpool.tile([P, NHALF], F32, tag="wmb")
                src = wmd_ap[e, ds(half * NHALF, NHALF)]
                src_b = bass.AP(
                    tensor=src.tensor, offset=src.offset, ap=[[0, P], *src.ap]
                )
                nc.gpsimd.dma_start(wmb, src_b)
                eot = eold.tile([P, CC, D], F32R, tag="eot")
                nc.sync.dma_start(eot, eod_ap[e].rearrange("c p d -> p c d"))
                C_e = cpool.tile([P, CC, NHALF], F32R, tag="C")
                for cc in range(CC):
                    nc.vector.tensor_mul(
                        C_e[:, cc, :], oh_T[:, cc, ds(half * NHALF, NHALF)], wmb
                    )
                for i in range(TC // 2):
                    for cc in range(CC):
                        nc.tensor.matmul(
                            psO[i],
                            lhsT=C_e[:, cc, ts(i, P)],
                            rhs=eot[:, cc, :],
                            start=(e == 0 and cc == 0),
                            stop=(e == E - 1 and cc == CC - 1),
                        )
            for i in range(TC // 2):
                o_sb = ostage.tile([P, D], F32, tag="osb")
                nc.scalar.copy(o_sb, psO[i])
                nc.sync.dma_start(out_t[half * (TC // 2) + i], o_sb)

    ctx.close()


_PROGRAM_CACHE = {}


def _get_program():
    if "nc" not in _PROGRAM_CACHE:
        _PROGRAM_CACHE["nc"] = build_program()
    return _PROGRAM_CACHE["nc"]


def _prep_weights(w1, w2):
    # w1s[e, hc, p, dc, hp] = w1[e, dc*128+p, hc*128+hp]
    w1s = np.ascontiguousarray(
        w1.reshape(E, DC, P, HC, P).transpose(0, 3, 2, 1, 4)
    ).astype(BF16_NP)
    w2b = w2.astype(BF16_NP)
    return w1s, w2b


def _run(x, w_gating, w1, w2, trace=False, **kwargs):
    from concourse.bass_utils import run_bass_kernel_spmd

    nc = _get_program()
    x = np.asarray(x, dtype=np.float32)
    wg = np.ascontiguousarray(np.asarray(w_gating, dtype=np.float32))
    w1s, w2b = _prep_weights(
        np.asarray(w1, dtype=np.float32), np.asarray(w2, dtype=np.float32)
    )
    in_maps = [
        {
            "x": np.ascontiguousarray(x[b]),
            "wg": wg,
            "w1s": w1s,
            "w2b": w2b,
        }
        for b in range(NCORES)
    ]
    res = run_bass_kernel_spmd(nc, in_maps, list(range(NCORES)), trace=trace, **kwargs)
    out = np.stack([res.results[b]["out"] for b in range(NCORES)], axis=0)
    lps = np.array(
        [np.float64(res.results[b]["lossp"].reshape(())) for b in range(NCORES)]
    )
    loss = np.float32(lps.sum() * E * LOSS_COEF / (float(N) * N * B))
    return (out, loss), res


def kernel(x, w_gating, w1, w2):
    (out, loss), _ = _run(x, w_gating, w1, w2, trace=False)
    return out, loss


if __name__ == "__main__":
    nc = _get_program()
    print("program built and compiled OK")


# revision 16
# speedup vs baseline: 1.0245x; 1.0245x over previous
"""MoE (threshold top-k routing, eval capacity) Trainium2 Bass kernel.

Strategy: data-parallel over the batch dim b (B=8 -> one batch element per
NeuronCore), full expert set computed locally on each core (no collectives).

Per-core program (N=2048 tokens, D=512, E=16, HID=2048, CAP=256):
  1. Gating logits via fp32 PE matmul (precision-critical: routing threshold
     margins are ~3e-5, so the gate path stays fp32 end-to-end).
  2. Softmax on ScalarE/VectorE; threshold top-k WITHOUT sorting via pairwise
     prob comparisons (expert e selected iff sum of probs strictly greater --
     ties broken by index -- is < 0.8).  Bit-matches jnp argsort semantics.
  3. Capacity: exclusive cumsum over tokens via the DVE scan instruction on an
     expert-major [16, 2048] layout; summed-slot quirk (pos_tok = sum over
     experts) reproduced exactly.
  4. Dispatch/combine as one-hot matmuls (handles duplicate-slot collisions by
     summation, exactly like the reference einsum).
  5. Expert FFN (gelu) with bf16 weights x float32r activations, fp32 PSUM.
"""

import sys

import numpy as np

sys.path.insert(0, "/opt/trn_rl_repo")

import ml_dtypes  # noqa: E402

import concourse.bass as bass  # noqa: E402
import concourse.mybir as mybir  # noqa: E402
import concourse.tile as tile  # noqa: E402
from concourse import bacc  # noqa: E402
from concourse.bass import ds, ts  # noqa: E402
from concourse.masks import make_identity  # noqa: E402

F32 = mybir.dt.float32
F32R = mybir.dt.float32r
BF16 = mybir.dt.bfloat16
I32 = mybir.dt.int32
AX = mybir.AxisListType
OP = mybir.AluOpType
AF = mybir.ActivationFunctionType

P = 128
B, N, D, E, HID, CAP = 8, 2048, 512, 16, 2048, 256
TC, DC, HC, CC = N // P, D // P, HID // P, CAP // P
NHALF = N // 2
THRESH = 0.8
LOSS_COEF = 0.01
NCORES = 8

BF16_NP = ml_dtypes.bfloat16


def _r(ap):
    """View an fp32 AP as float32r for 1-cycle/row PE matmuls."""
    return ap.bitcast(F32R)


def build_program():
    nc = bacc.Bacc(
        "TRN2",
        target_bir_lowering=False,
        debug=False,
        enable_asserts=False,
        num_devices=1,
    )

    x_ap = nc.dram_tensor("x", [N, D], F32, kind="ExternalInput").ap()
    wg_ap = nc.dram_tensor("wg", [D, E], F32, kind="ExternalInput").ap()
    # w1s[e, g, p, hh, dc, hp] = w1[e, dc*128+p, (g*8+hh)*128+hp]  (host-prepped)
    w1_ap = nc.dram_tensor("w1s", [E, 2, P, HC // 2, DC, P], BF16, kind="ExternalInput").ap()
    # w2s[e, g, p, hh, d] = w2[e, (g*8+hh)*128+p, d]  (host-prepped)
    w2_ap = nc.dram_tensor("w2s", [E, 2, P, HC // 2, D], BF16, kind="ExternalInput").ap()
    out_ap = nc.dram_tensor("out", [N, D], F32, kind="ExternalOutput").ap()
    lossp_ap = nc.dram_tensor("lossp", [1, 1], F32, kind="ExternalOutput").ap()
    # internal DRAM bounce buffers
    eod_ap = nc.dram_tensor("eod", [E, CC, P, D], F32R).ap()
    wmd_ap = nc.dram_tensor("wmd", [E, N], F32).ap()

    out_t = out_ap.rearrange("(t p) d -> t p d", p=P)

    with tile.TileContext(nc) as tcx:
        _emit(tcx, x_ap, wg_ap, w1_ap, w2_ap, out_t, lossp_ap, eod_ap, wmd_ap)

    nc.compile()
    return nc


def _emit(tcx, x_ap, wg_ap, w1_ap, w2_ap, out_t, lossp_ap, eod_ap, wmd_ap):
    from contextlib import ExitStack

    nc = tcx.nc
    ctx = ExitStack()
    const = ctx.enter_context(tcx.tile_pool(name="const", bufs=1))
    keep = ctx.enter_context(tcx.tile_pool(name="keep", bufs=1))
    ctxR = ExitStack()
    rp = ctxR.enter_context(tcx.tile_pool(name="routing", bufs=1))
    pw = ctxR.enter_context(tcx.tile_pool(name="pairwise", bufs=2))
    psT = ctxR.enter_context(tcx.tile_pool(name="psT", bufs=6, space="PSUM"))

    # ---------------- constants ----------------
    ident = const.tile([P, P], F32, name="ident")
    make_identity(nc, ident)
    ones16 = const.tile([E, 1], F32, name="ones16")
    nc.vector.memset(ones16, 1.0)
    iota_ci = const.tile([P, CAP], I32, name="iota_ci")
    nc.gpsimd.iota(iota_ci, pattern=[[1, CAP]], base=0, channel_multiplier=0)
    iota_c = const.tile([P, CAP], F32, name="iota_c")
    nc.vector.tensor_copy(iota_c, iota_ci)
    iota_pi = const.tile([P, CC], I32, name="iota_pi")
    nc.gpsimd.iota(iota_pi, pattern=[[P, CC]], base=0, channel_multiplier=1)
    iota_p = const.tile([P, CC], F32, name="iota_p")
    nc.vector.tensor_copy(iota_p, iota_pi)
    # tri[p, ep, e] = 1.0 where e > ep else 0.0 (tie-break mask)
    tri = const.tile([P, E, E], F32, name="tri")
    nc.gpsimd.memset(tri, 1.0)
    nc.gpsimd.affine_select(
        out=tri, in_=tri, pattern=[[-1, E], [1, E]], base=0,
        channel_multiplier=0, compare_op=OP.is_gt, fill=0.0,
    )

    # ---------------- load x, wg ----------------
    x_sb = rp.tile([P, TC, D], F32, name="x_sb")
    nc.sync.dma_start(x_sb, x_ap.rearrange("(t p) d -> p t d", p=P))
    wg_sb = rp.tile([P, DC, E], F32, name="wg_sb")
    nc.sync.dma_start(wg_sb, wg_ap.rearrange("(c p) e -> p c e", p=P))

    # ---------------- transpose x -> xT [d-part, dc, t] ----------------
    with tcx.tile_pool(name="xt", bufs=1) as xtp:
        xT = xtp.tile([P, DC, N], F32, name="xT")
        for t in range(TC):
            for dc in range(DC):
                ps = psT.tile([P, P], F32, tag="ps")
                nc.tensor.transpose(ps, x_sb[:, t, ds(dc * P, P)], ident)
                nc.vector.tensor_copy(xT[:, dc, ds(t * P, P)], ps)

        # ---------------- gating logits [t-part, tc, e] (full fp32) ------
        lg = rp.tile([P, TC, E], F32, name="lg")
        for t in range(TC):
            ps = psT.tile([P, E], F32, tag="ps")
            for dc in range(DC):
                nc.tensor.matmul(
                    ps, lhsT=xT[:, dc, ts(t, P)], rhs=wg_sb[:, dc, :],
                    start=(dc == 0), stop=(dc == DC - 1),
                )
            nc.vector.tensor_copy(lg[:, t, :], ps)

    # persistent f32r copy of x for the dispatch matmuls (after xT freed)
    ctxX = ExitStack()
    xrp = ctxX.enter_context(tcx.tile_pool(name="xrp", bufs=1, side="right"))
    x_r = xrp.tile([P, TC, D], BF16, name="x_r")
    nc.vector.tensor_copy(x_r, x_sb)

    # ---------------- softmax (fp32) ----------------
    rmax = rp.tile([P, TC], F32, name="rmax")
    nc.vector.reduce_max(rmax, lg, axis=AX.X)
    sh = rp.tile([P, TC, E], F32, name="sh")
    nc.vector.tensor_sub(sh, lg, rmax[:, :, None].to_broadcast([P, TC, E]))
    u = rp.tile([P, TC, E], F32, name="u")
    nc.scalar.activation(u, sh, AF.Exp)
    usum = rp.tile([P, TC], F32, name="usum")
    nc.vector.reduce_sum(usum, u, axis=AX.X)
    rin = rp.tile([P, TC], F32, name="rin")
    nc.vector.reciprocal(rin, usum)
    p_sb = rp.tile([P, TC, E], F32, name="p_sb")
    nc.vector.tensor_mul(p_sb, u, rin[:, :, None].to_broadcast([P, TC, E]))

    # ------------- threshold top-k via pairwise comparisons -------------
    # S[t, e] = sum_{e'} p[t,e'] * [ p_e' > p_e  or (p_e' == p_e and e' < e) ]
    # mask[t, e] = S[t, e] < THRESH      (== reference argsort/cumsum mask)
    S = rp.tile([P, TC, E], F32, name="S")
    nc.vector.memset(S, 0.0)
    for ep in range(E):
        pb = p_sb[:, :, ep : ep + 1].to_broadcast([P, TC, E])
        gt = pw.tile([P, TC, E], F32, tag="gt")
        nc.vector.tensor_tensor(gt, pb, p_sb, OP.is_gt)
        eq = pw.tile([P, TC, E], F32, tag="eq")
        nc.vector.tensor_tensor(eq, pb, p_sb, OP.is_equal)
        # ties only count for columns e > ep
        m = pw.tile([P, TC, E], F32, tag="m")
        nc.vector.tensor_mul(m, eq, tri[:, ep : ep + 1, :].to_broadcast([P, TC, E]))
        nc.vector.tensor_add(m, m, gt)
        t2 = pw.tile([P, TC, E], F32, tag="t2")
        nc.vector.tensor_mul(t2, m, pb)
        nc.vector.tensor_add(S, S, t2)

    mask = rp.tile([P, TC, E], F32, name="mask")
    nc.vector.tensor_single_scalar(mask, S, THRESH, OP.is_lt)
    selp = rp.tile([P, TC, E], F32, name="selp")
    nc.vector.tensor_mul(selp, p_sb, mask)
    wsum = rp.tile([P, TC], F32, name="wsum")
    nc.vector.reduce_sum(wsum, selp, axis=AX.X)
    winv = rp.tile([P, TC], F32, name="winv")
    nc.vector.reciprocal(winv, wsum)
    wts = rp.tile([P, TC, E], F32, name="wts")
    nc.vector.tensor_mul(wts, selp, winv[:, :, None].to_broadcast([P, TC, E]))

    # ------------- transpose mask/wts/p to expert-major [E, N] -------------
    mask_T = rp.tile([E, N], F32, name="mask_T")
    wts_T = rp.tile([E, N], F32, name="wts_T")
    p_T = rp.tile([E, N], F32, name="p_T")
    for t in range(TC):
        for src, dst in ((mask, mask_T), (wts, wts_T), (p_sb, p_T)):
            ps = psT.tile([E, P], F32, tag="ps")
            nc.tensor.transpose(ps, src[:, t, :], ident)
            nc.vector.tensor_copy(dst[:, ds(t * P, P)], ps)

    # ------------- capacity (exclusive cumsum over tokens) -------------
    cum = rp.tile([E, N], F32, name="cum")
    nc.vector.tensor_tensor_scan(cum, mask_T, mask_T, 0.0, OP.add, OP.bypass)
    pos = rp.tile([E, N], F32, name="pos")
    nc.vector.tensor_sub(pos, cum, mask_T)
    mask2_T = rp.tile([E, N], F32, name="mask2_T")
    nc.vector.scalar_tensor_tensor(
        mask2_T, in0=pos, scalar=float(CAP), in1=mask_T, op0=OP.is_lt, op1=OP.mult
    )
    pos2 = rp.tile([E, N], F32, name="pos2")
    nc.vector.tensor_mul(pos2, pos, mask2_T)
    wm_T = rp.tile([E, N], F32, name="wm_T")
    nc.vector.tensor_mul(wm_T, wts_T, mask2_T)
    nc.sync.dma_start(wmd_ap, wm_T)

    # ------------- aux loss partial: sum_e mean_t(p) * mean_t(mask2) -----
    proxs = rp.tile([E, 1], F32, name="proxs")
    nc.vector.reduce_sum(proxs, p_T, axis=AX.X)
    denss = rp.tile([E, 1], F32, name="denss")
    nc.vector.reduce_sum(denss, mask2_T, axis=AX.X)
    prod = rp.tile([E, 1], F32, name="prod")
    nc.vector.tensor_mul(prod, proxs, denss)
    psl = psT.tile([1, 1], F32, tag="ps")
    nc.tensor.matmul(psl, lhsT=prod, rhs=ones16, start=True, stop=True)
    lp_sb = rp.tile([1, 1], F32, name="lp_sb")
    nc.scalar.copy(lp_sb, psl)
    nc.sync.dma_start(lossp_ap, lp_sb)

    # ------------- summed slot index s(t) = sum_e pos2  -------------
    # expert-major broadcast copy: s_bcast[c-part, t]
    s_row = rp.tile([1, N], F32, name="s_row")
    for q in range(N // 512):
        ps = psT.tile([1, 512], F32, tag="ps")
        nc.tensor.matmul(
            ps, lhsT=ones16, rhs=pos2[:, ds(q * 512, 512)], start=True, stop=True
        )
        nc.vector.tensor_copy(s_row[:, ds(q * 512, 512)], ps)
    ones1 = const.tile([1, P], F32, name="ones1")
    nc.vector.memset(ones1, 1.0)
    s_bcast = rp.tile([P, N], F32, name="s_bcast")
    for q in range(N // 512):
        ps = psT.tile([P, 512], F32, tag="ps")
        nc.tensor.matmul(
            ps, lhsT=ones1, rhs=s_row[:, ds(q * 512, 512)], start=True, stop=True
        )
        nc.vector.tensor_copy(s_bcast[:, ds(q * 512, 512)], ps)

    # token-major mask2 and s: transpose back
    mask2_tok = keep.tile([P, TC, E], F32, name="mask2_tok")
    pos2_tok = rp.tile([P, TC, E], F32, name="pos2_tok")
    for t in range(TC):
        for src, dst in ((mask2_T, mask2_tok), (pos2, pos2_tok)):
            ps = psT.tile([P, E], F32, tag="ps")
            nc.tensor.transpose(ps, src[:, ds(t * P, P)], ident[:E, :E])
            nc.vector.tensor_copy(dst[:, t, :], ps)
    s_tok = rp.tile([P, TC], F32, name="s_tok")
    nc.vector.reduce_sum(s_tok, pos2_tok, axis=AX.X)

    # ------------- one-hot slot structures -------------
    oh_tok = keep.tile([P, TC, CAP], BF16, name="oh_tok")  # [t, c] = (s(t)==c)
    nc.vector.tensor_tensor(
        oh_tok,
        s_tok[:, :, None].to_broadcast([P, TC, CAP]),
        iota_c[:, None, :].to_broadcast([P, TC, CAP]),
        OP.is_equal,
    )
    oh_T = keep.tile([P, CC, N], BF16, name="oh_T")  # [c, t] = (s(t)==c)
    for cc in range(CC):
        nc.vector.tensor_tensor(
            oh_T[:, cc, :],
            s_bcast,
            iota_p[:, cc : cc + 1].to_broadcast([P, N]),
            OP.is_equal,
        )

    ctxR.close()

    # ================= Phase A: dispatch + expert FFN =================
    with (
        tcx.tile_pool(name="mpool", bufs=2) as mpool,
        tcx.tile_pool(name="w1pool", bufs=2) as w1pool,
        tcx.tile_pool(name="w2pool", bufs=2) as w2pool,
        tcx.tile_pool(name="eipool", bufs=2) as eipool,
        tcx.tile_pool(name="hpool", bufs=2) as hpool,
        tcx.tile_pool(name="eopool", bufs=3) as eopool,
        tcx.tile_pool(name="psA", bufs=1, space="PSUM") as psA,
        tcx.tile_pool(name="psH", bufs=2, space="PSUM") as psH,
    ):
        for e in range(E):
            # M_e[t, c] = oh[t, c] * mask2[t, e]   (bf16, exact 0/1 weights)
            M_e = mpool.tile([P, TC, CAP], BF16, tag="M")
            for t in range(TC):
                nc.vector.tensor_scalar_mul(
                    M_e[:, t, :], oh_tok[:, t, :], mask2_tok[:, t, e : e + 1]
                )
            # EI_T[d, c] = sum_t x[t, d] * M_e[t, c]
            ei = eipool.tile([P, DC, CAP], BF16, tag="ei")
            for dc in range(DC):
                pse = psA.tile([P, CAP], F32, tag=f"ei{dc}")
                for t in range(TC):
                    nc.tensor.matmul(
                        pse,
                        lhsT=x_r[:, t, ds(dc * P, P)],
                        rhs=M_e[:, t, :],
                        start=(t == 0),
                        stop=(t == TC - 1),
                    )
                nc.scalar.copy(ei[:, dc, :], pse)
            # h[hid, c] = gelu(sum_d w1[d, hid] * EI_T[d, c])
            h = hpool.tile([P, HC, CAP], BF16, tag="h")
            for g in range(2):
                w1t = w1pool.tile([P, HC // 2, DC, P], BF16, tag="w1")
                nc.sync.dma_start(w1t, w1_ap[e, g])
                for hh in range(HC // 2):
                    psh = psH.tile([P, CAP], F32, tag="h")
                    for dc in range(DC):
                        nc.tensor.matmul(
                            psh,
                            lhsT=w1t[:, hh, dc, :],
                            rhs=ei[:, dc, :],
                            start=(dc == 0),
                            stop=(dc == DC - 1),
                        )
                    nc.scalar.activation(h[:, g * (HC // 2) + hh, :], psh, AF.Gelu)
            # EO[c, d] = sum_hid h[hid, c] * w2[hid, d]
            pso = [
                psA.tile([P, D], F32, tag=f"eo{cc}", name=f"pso{cc}")
                for cc in range(CC)
            ]
            for g in range(2):
                w2t = w2pool.tile([P, HC // 2, D], BF16, tag="w2")
                nc.sync.dma_start(w2t, w2_ap[e, g])
                for hh in range(HC // 2):
                    hc = g * (HC // 2) + hh
                    for cc in range(CC):
                        nc.tensor.matmul(
                            pso[cc],
                            lhsT=h[:, hc, ds(cc * P, P)],
                            rhs=w2t[:, hh, :],
                            start=(hc == 0),
                            stop=(hc == HC - 1),
                        )
            for cc in range(CC):
                eo_sb = eopool.tile([P, D], F32R, tag="eo")
                nc.scalar.copy(eo_sb, pso[cc])
                nc.sync.dma_start(eod_ap[e, cc], eo_sb)

    ctxX.close()

    # ================= Phase B: combine =================
    with (
        tcx.tile_pool(name="cpool", bufs=2) as cpool,
        tcx.tile_pool(name="wmpool", bufs=2) as wmpool,
        tcx.tile_pool(name="eold", bufs=2) as eold,
        tcx.tile_pool(name="ostage", bufs=3) as ostage,
        tcx.tile_pool(name="psB", bufs=1, space="PSUM") as psB,
    ):
        for half in range(2):
            psO = [
                psB.tile([P, D], F32, tag=f"o{i}", name=f"psO{i}")
                for i in range(TC // 2)
            ]
            for e in range(E):
                wmb = wmpool.tile([P, NHALF], F32, tag="wmb")
                src = wmd_ap[e, ds(half * NHALF, NHALF)]
                src_b = bass.AP(
                    tensor=src.tensor, offset=src.offset, ap=[[0, P], *src.ap]
                )
                nc.gpsimd.dma_start(wmb, src_b)
                eot = eold.tile([P, CC, D], F32R, tag="eot")
                nc.sync.dma_start(eot, eod_ap[e].rearrange("c p d -> p c d"))
                C_e = cpool.tile([P, CC, NHALF], F32R, tag="C")
                for cc in range(CC):
                    nc.vector.tensor_mul(
                        C_e[:, cc, :], oh_T[:, cc, ds(half * NHALF, NHALF)], wmb
                    )
                for i in range(TC // 2):
                    for cc in range(CC):
                        nc.tensor.matmul(
                            psO[i],
                            lhsT=C_e[:, cc, ts(i, P)],
                            rhs=eot[:, cc, :],
                            start=(e == 0 and cc == 0),
                            stop=(e == E - 1 and cc == CC - 1),
                        )
            for i in range(TC // 2):
                o_sb = ostage.tile([P, D], F32, tag="osb")
                nc.scalar.copy(o_sb, psO[i])
                nc.sync.dma_start(out_t[half * (TC // 2) + i], o_sb)

    ctx.close()


_PROGRAM_CACHE = {}


def _get_program():
    if "nc" not in _PROGRAM_CACHE:
        _PROGRAM_CACHE["nc"] = build_program()
    return _PROGRAM_CACHE["nc"]


def _prep_weights(w1, w2):
    # w1s[e, g, p, hh, dc, hp] = w1[e, dc*128+p, (g*8+hh)*128+hp]
    w1s = np.ascontiguousarray(
        w1.reshape(E, DC, P, 2, HC // 2, P).transpose(0, 3, 2, 4, 1, 5)
    ).astype(BF16_NP)
    # w2s[e, g, p, hh, d] = w2[e, (g*8+hh)*128+p, d]
    w2s = np.ascontiguousarray(
        w2.reshape(E, 2, HC // 2, P, D).transpose(0, 1, 3, 2, 4)
    ).astype(BF16_NP)
    return w1s, w2s


def _run(x, w_gating, w1, w2, trace=False, **kwargs):
    from concourse.bass_utils import run_bass_kernel_spmd

    nc = _get_program()
    x = np.asarray(x, dtype=np.float32)
    wg = np.ascontiguousarray(np.asarray(w_gating, dtype=np.float32))
    w1s, w2s = _prep_weights(
        np.asarray(w1, dtype=np.float32), np.asarray(w2, dtype=np.float32)
    )
    in_maps = [
        {
            "x": np.ascontiguousarray(x[b]),
            "wg": wg,
            "w1s": w1s,
            "w2s": w2s,
        }
        for b in range(NCORES)
    ]
    res = run_bass_kernel_spmd(nc, in_maps, list(range(NCORES)), trace=trace, **kwargs)
    out = np.stack([res.results[b]["out"] for b in range(NCORES)], axis=0)
    lps = np.array(
        [np.float64(res.results[b]["lossp"].reshape(())) for b in range(NCORES)]
    )
    loss = np.float32(lps.sum() * E * LOSS_COEF / (float(N) * N * B))
    return (out, loss), res


def kernel(x, w_gating, w1, w2):
    (out, loss), _ = _run(x, w_gating, w1, w2, trace=False)
    return out, loss


if __name__ == "__main__":
    nc = _get_program()
    print("program built and compiled OK")


# revision 18
# speedup vs baseline: 1.1792x; 1.1509x over previous
"""MoE (threshold top-k routing, eval capacity) Trainium2 Bass kernel.

Strategy: data-parallel over the batch dim b (B=8 -> one batch element per
NeuronCore), full expert set computed locally on each core (no collectives).

Per-core program (N=2048 tokens, D=512, E=16, HID=2048, CAP=256):
  1. Gating logits via fp32 PE matmul (precision-critical: routing threshold
     margins are ~3e-5, so the gate path stays fp32 end-to-end).
  2. Softmax on ScalarE/VectorE; threshold top-k WITHOUT sorting via pairwise
     prob comparisons (expert e selected iff sum of probs strictly greater --
     ties broken by index -- is < 0.8).  Bit-matches jnp argsort semantics.
  3. Capacity: exclusive cumsum over tokens via the DVE scan instruction on an
     expert-major [16, 2048] layout; summed-slot quirk (pos_tok = sum over
     experts) reproduced exactly.
  4. Dispatch/combine as one-hot matmuls (handles duplicate-slot collisions by
     summation, exactly like the reference einsum).
  5. Expert FFN (gelu) with bf16 weights x float32r activations, fp32 PSUM.
"""

import sys

import numpy as np

sys.path.insert(0, "/opt/trn_rl_repo")

import ml_dtypes  # noqa: E402

import concourse.bass as bass  # noqa: E402
import concourse.mybir as mybir  # noqa: E402
import concourse.tile as tile  # noqa: E402
from concourse import bacc  # noqa: E402
from concourse.bass import ds, ts  # noqa: E402
from concourse.masks import make_identity  # noqa: E402

F32 = mybir.dt.float32
F32R = mybir.dt.float32r
BF16 = mybir.dt.bfloat16
I32 = mybir.dt.int32
AX = mybir.AxisListType
OP = mybir.AluOpType
AF = mybir.ActivationFunctionType

P = 128
B, N, D, E, HID, CAP = 8, 2048, 512, 16, 2048, 256
TC, DC, HC, CC = N // P, D // P, HID // P, CAP // P
NHALF = N // 2
THRESH = 0.8
LOSS_COEF = 0.01
NCORES = 8

BF16_NP = ml_dtypes.bfloat16


def _r(ap):
    """View an fp32 AP as float32r for 1-cycle/row PE matmuls."""
    return ap.bitcast(F32R)


def build_program():
    nc = bacc.Bacc(
        "TRN2",
        target_bir_lowering=False,
        debug=False,
        enable_asserts=False,
        num_devices=1,
    )

    x_ap = nc.dram_tensor("x", [N, D], F32, kind="ExternalInput").ap()
    wg_ap = nc.dram_tensor("wg", [D, E], F32, kind="ExternalInput").ap()
    # w1s[e, g, p, hh, dc, hp] = w1[e, dc*128+p, (g*8+hh)*128+hp]  (host-prepped)
    w1_ap = nc.dram_tensor("w1s", [E, 2, P, HC // 2, DC, P], BF16, kind="ExternalInput").ap()
    # w2s[e, g, p, hh, d] = w2[e, (g*8+hh)*128+p, d]  (host-prepped)
    w2_ap = nc.dram_tensor("w2s", [E, 2, P, HC // 2, D], BF16, kind="ExternalInput").ap()
    out_ap = nc.dram_tensor("out", [N, D], F32, kind="ExternalOutput").ap()
    lossp_ap = nc.dram_tensor("lossp", [1, 1], F32, kind="ExternalOutput").ap()
    # internal DRAM bounce buffers
    eod_ap = nc.dram_tensor("eod", [E, CC, P, D], F32R).ap()
    wmd_ap = nc.dram_tensor("wmd", [E, N], F32).ap()

    out_t = out_ap.rearrange("(t p) d -> t p d", p=P)

    with tile.TileContext(nc) as tcx:
        _emit(tcx, x_ap, wg_ap, w1_ap, w2_ap, out_t, lossp_ap, eod_ap, wmd_ap)

    nc.compile()
    return nc


def _emit(tcx, x_ap, wg_ap, w1_ap, w2_ap, out_t, lossp_ap, eod_ap, wmd_ap):
    from contextlib import ExitStack

    nc = tcx.nc
    ctx = ExitStack()
    const = ctx.enter_context(tcx.tile_pool(name="const", bufs=1))
    keep = ctx.enter_context(tcx.tile_pool(name="keep", bufs=1))
    ctxR = ExitStack()
    rp = ctxR.enter_context(tcx.tile_pool(name="routing", bufs=1))
    pw = ctxR.enter_context(tcx.tile_pool(name="pairwise", bufs=2))
    psT = ctxR.enter_context(tcx.tile_pool(name="psT", bufs=6, space="PSUM"))

    # ---------------- constants ----------------
    ident = const.tile([P, P], F32, name="ident")
    make_identity(nc, ident)
    ones16 = const.tile([E, 1], F32, name="ones16")
    nc.vector.memset(ones16, 1.0)
    iota_ci = const.tile([P, CAP], I32, name="iota_ci")
    nc.gpsimd.iota(iota_ci, pattern=[[1, CAP]], base=0, channel_multiplier=0)
    iota_c = const.tile([P, CAP], F32, name="iota_c")
    nc.vector.tensor_copy(iota_c, iota_ci)
    iota_pi = const.tile([P, CC], I32, name="iota_pi")
    nc.gpsimd.iota(iota_pi, pattern=[[P, CC]], base=0, channel_multiplier=1)
    iota_p = const.tile([P, CC], F32, name="iota_p")
    nc.vector.tensor_copy(iota_p, iota_pi)
    # tri[p, ep, e] = 1.0 where e > ep else 0.0 (tie-break mask)
    tri = const.tile([P, E, E], F32, name="tri")
    nc.gpsimd.memset(tri, 1.0)
    nc.gpsimd.affine_select(
        out=tri, in_=tri, pattern=[[-1, E], [1, E]], base=0,
        channel_multiplier=0, compare_op=OP.is_gt, fill=0.0,
    )

    # ---------------- load x, wg ----------------
    x_sb = rp.tile([P, TC, D], F32, name="x_sb")
    nc.sync.dma_start(x_sb, x_ap.rearrange("(t p) d -> p t d", p=P))
    wg_sb = rp.tile([P, DC, E], F32, name="wg_sb")
    nc.sync.dma_start(wg_sb, wg_ap.rearrange("(c p) e -> p c e", p=P))

    # ---------------- transpose x -> xT [d-part, dc, t] ----------------
    with tcx.tile_pool(name="xt", bufs=1) as xtp:
        xT = xtp.tile([P, DC, N], F32, name="xT")
        for t in range(TC):
            for dc in range(DC):
                ps = psT.tile([P, P], F32, tag="ps")
                nc.tensor.transpose(ps, x_sb[:, t, ds(dc * P, P)], ident)
                nc.scalar.copy(xT[:, dc, ds(t * P, P)], ps)

        # ---------------- gating logits [t-part, tc, e] (full fp32) ------
        lg = rp.tile([P, TC, E], F32, name="lg")
        for t in range(TC):
            ps = psT.tile([P, E], F32, tag="ps")
            for dc in range(DC):
                nc.tensor.matmul(
                    ps, lhsT=xT[:, dc, ts(t, P)], rhs=wg_sb[:, dc, :],
                    start=(dc == 0), stop=(dc == DC - 1),
                )
            nc.scalar.copy(lg[:, t, :], ps)

    # persistent f32r copy of x for the dispatch matmuls (after xT freed)
    ctxX = ExitStack()
    xrp = ctxX.enter_context(tcx.tile_pool(name="xrp", bufs=1, side="right"))
    x_r = xrp.tile([P, TC, D], BF16, name="x_r")
    nc.vector.tensor_copy(x_r, x_sb)

    # ---------------- softmax (fp32) ----------------
    rmax = rp.tile([P, TC], F32, name="rmax")
    nc.vector.reduce_max(rmax, lg, axis=AX.X)
    sh = rp.tile([P, TC, E], F32, name="sh")
    nc.vector.tensor_sub(sh, lg, rmax[:, :, None].to_broadcast([P, TC, E]))
    u = rp.tile([P, TC, E], F32, name="u")
    nc.scalar.activation(u, sh, AF.Exp)
    usum = rp.tile([P, TC], F32, name="usum")
    nc.vector.reduce_sum(usum, u, axis=AX.X)
    rin = rp.tile([P, TC], F32, name="rin")
    nc.vector.reciprocal(rin, usum)
    p_sb = rp.tile([P, TC, E], F32, name="p_sb")
    nc.vector.tensor_mul(p_sb, u, rin[:, :, None].to_broadcast([P, TC, E]))

    # ------------- threshold top-k via pairwise comparisons -------------
    # S[t, e] = sum_{e'} p[t,e'] * [ p_e' > p_e  or (p_e' == p_e and e' < e) ]
    # mask[t, e] = S[t, e] < THRESH      (== reference argsort/cumsum mask)
    S = rp.tile([P, TC, E], F32, name="S")
    nc.vector.memset(S, 0.0)
    for ep in range(E):
        eng = nc.vector
        acc = S
        sfx = ""
        pb = p_sb[:, :, ep : ep + 1].to_broadcast([P, TC, E])
        gt = pw.tile([P, TC, E], F32, tag="gt" + sfx)
        eng.tensor_tensor(gt, pb, p_sb, OP.is_gt)
        eq = pw.tile([P, TC, E], F32, tag="eq" + sfx)
        eng.tensor_tensor(eq, pb, p_sb, OP.is_equal)
        # ties only count for columns e > ep
        m = pw.tile([P, TC, E], F32, tag="m" + sfx)
        eng.tensor_mul(m, eq, tri[:, ep : ep + 1, :].to_broadcast([P, TC, E]))
        eng.tensor_add(m, m, gt)
        t2 = pw.tile([P, TC, E], F32, tag="t2" + sfx)
        eng.tensor_mul(t2, m, pb)
        eng.tensor_add(acc, acc, t2)

    mask = rp.tile([P, TC, E], F32, name="mask")
    nc.vector.tensor_single_scalar(mask, S, THRESH, OP.is_lt)
    selp = rp.tile([P, TC, E], F32, name="selp")
    nc.vector.tensor_mul(selp, p_sb, mask)
    wsum = rp.tile([P, TC], F32, name="wsum")
    nc.vector.reduce_sum(wsum, selp, axis=AX.X)
    winv = rp.tile([P, TC], F32, name="winv")
    nc.vector.reciprocal(winv, wsum)
    wts = rp.tile([P, TC, E], F32, name="wts")
    nc.vector.tensor_mul(wts, selp, winv[:, :, None].to_broadcast([P, TC, E]))

    # ------------- transpose mask/wts/p to expert-major [E, N] -------------
    mask_T = rp.tile([E, N], F32, name="mask_T")
    wts_T = rp.tile([E, N], F32, name="wts_T")
    p_T = rp.tile([E, N], F32, name="p_T")
    for t in range(TC):
        for src, dst in ((mask, mask_T), (wts, wts_T), (p_sb, p_T)):
            ps = psT.tile([E, P], F32, tag="ps")
            nc.tensor.transpose(ps, src[:, t, :], ident)
            nc.scalar.copy(dst[:, ds(t * P, P)], ps)

    # ------------- capacity (exclusive cumsum over tokens) -------------
    cum = rp.tile([E, N], F32, name="cum")
    nc.vector.tensor_tensor_scan(cum, mask_T, mask_T, 0.0, OP.add, OP.bypass)
    pos = rp.tile([E, N], F32, name="pos")
    nc.vector.tensor_sub(pos, cum, mask_T)
    mask2_T = rp.tile([E, N], F32, name="mask2_T")
    nc.vector.scalar_tensor_tensor(
        mask2_T, in0=pos, scalar=float(CAP), in1=mask_T, op0=OP.is_lt, op1=OP.mult
    )
    pos2 = rp.tile([E, N], F32, name="pos2")
    nc.vector.tensor_mul(pos2, pos, mask2_T)
    wm_T = rp.tile([E, N], F32, name="wm_T")
    nc.vector.tensor_mul(wm_T, wts_T, mask2_T)
    nc.sync.dma_start(wmd_ap, wm_T)

    # ------------- aux loss partial: sum_e mean_t(p) * mean_t(mask2) -----
    proxs = rp.tile([E, 1], F32, name="proxs")
    nc.vector.reduce_sum(proxs, p_T, axis=AX.X)
    denss = rp.tile([E, 1], F32, name="denss")
    nc.vector.reduce_sum(denss, mask2_T, axis=AX.X)
    prod = rp.tile([E, 1], F32, name="prod")
    nc.vector.tensor_mul(prod, proxs, denss)
    psl = psT.tile([1, 1], F32, tag="ps")
    nc.tensor.matmul(psl, lhsT=prod, rhs=ones16, start=True, stop=True)
    lp_sb = rp.tile([1, 1], F32, name="lp_sb")
    nc.scalar.copy(lp_sb, psl)
    nc.sync.dma_start(lossp_ap, lp_sb)

    # ------------- summed slot index s(t) = sum_e pos2  -------------
    # expert-major broadcast copy: s_bcast[c-part, t]
    s_row = rp.tile([1, N], F32, name="s_row")
    for q in range(N // 512):
        ps = psT.tile([1, 512], F32, tag="ps")
        nc.tensor.matmul(
            ps, lhsT=ones16, rhs=pos2[:, ds(q * 512, 512)], start=True, stop=True
        )
        nc.scalar.copy(s_row[:, ds(q * 512, 512)], ps)
    ones1 = const.tile([1, P], F32, name="ones1")
    nc.vector.memset(ones1, 1.0)
    s_bcast = rp.tile([P, N], F32, name="s_bcast")
    for q in range(N // 512):
        ps = psT.tile([P, 512], F32, tag="ps")
        nc.tensor.matmul(
            ps, lhsT=ones1, rhs=s_row[:, ds(q * 512, 512)], start=True, stop=True
        )
        nc.scalar.copy(s_bcast[:, ds(q * 512, 512)], ps)

    # token-major mask2 and s: transpose back
    mask2_tok = keep.tile([P, TC, E], F32, name="mask2_tok")
    pos2_tok = rp.tile([P, TC, E], F32, name="pos2_tok")
    for t in range(TC):
        for src, dst in ((mask2_T, mask2_tok), (pos2, pos2_tok)):
            ps = psT.tile([P, E], F32, tag="ps")
            nc.tensor.transpose(ps, src[:, ds(t * P, P)], ident[:E, :E])
            nc.scalar.copy(dst[:, t, :], ps)
    s_tok = rp.tile([P, TC], F32, name="s_tok")
    nc.vector.reduce_sum(s_tok, pos2_tok, axis=AX.X)

    # ------------- one-hot slot structures -------------
    oh_tok = keep.tile([P, TC, CAP], BF16, name="oh_tok")  # [t, c] = (s(t)==c)
    nc.vector.tensor_tensor(
        oh_tok,
        s_tok[:, :, None].to_broadcast([P, TC, CAP]),
        iota_c[:, None, :].to_broadcast([P, TC, CAP]),
        OP.is_equal,
    )
    oh_T = keep.tile([P, CC, N], BF16, name="oh_T")  # [c, t] = (s(t)==c)
    for cc in range(CC):
        nc.vector.tensor_tensor(
            oh_T[:, cc, :],
            s_bcast,
            iota_p[:, cc : cc + 1].to_broadcast([P, N]),
            OP.is_equal,
        )

    ctxR.close()

    # ================= Phase A: dispatch + expert FFN =================
    with (
        tcx.tile_pool(name="mpool", bufs=2) as mpool,
        tcx.tile_pool(name="w1pool", bufs=2) as w1pool,
        tcx.tile_pool(name="w2pool", bufs=2) as w2pool,
        tcx.tile_pool(name="eipool", bufs=2) as eipool,
        tcx.tile_pool(name="hpool", bufs=2) as hpool,
        tcx.tile_pool(name="eopool", bufs=3) as eopool,
        tcx.tile_pool(name="psA", bufs=1, space="PSUM") as psA,
        tcx.tile_pool(name="psH", bufs=2, space="PSUM") as psH,
    ):
        for e in range(E):
            # M_e[t, c] = oh[t, c] * mask2[t, e]   (bf16, exact 0/1 weights)
            M_e = mpool.tile([P, TC, CAP], BF16, tag="M")
            for t in range(TC):
                nc.vector.tensor_scalar_mul(
                    M_e[:, t, :], oh_tok[:, t, :], mask2_tok[:, t, e : e + 1]
                )
            # EI_T[d, c] = sum_t x[t, d] * M_e[t, c]
            ei = eipool.tile([P, DC, CAP], BF16, tag="ei")
            for dc in range(DC):
                pse = psA.tile([P, CAP], F32, tag=f"ei{dc}")
                for t in range(TC):
                    nc.tensor.matmul(
                        pse,
                        lhsT=x_r[:, t, ds(dc * P, P)],
                        rhs=M_e[:, t, :],
                        start=(t == 0),
                        stop=(t == TC - 1),
                    )
                nc.scalar.copy(ei[:, dc, :], pse)
            # h[hid, c] = gelu(sum_d w1[d, hid] * EI_T[d, c])
            h = hpool.tile([P, HC, CAP], BF16, tag="h")
            for g in range(2):
                w1t = w1pool.tile([P, HC // 2, DC, P], BF16, tag="w1")
                nc.sync.dma_start(w1t, w1_ap[e, g])
                for hh in range(HC // 2):
                    psh = psH.tile([P, CAP], F32, tag="h")
                    for dc in range(DC):
                        nc.tensor.matmul(
                            psh,
                            lhsT=w1t[:, hh, dc, :],
                            rhs=ei[:, dc, :],
                            start=(dc == 0),
                            stop=(dc == DC - 1),
                        )
                    nc.scalar.activation(h[:, g * (HC // 2) + hh, :], psh, AF.Gelu)
            # EO[c, d] = sum_hid h[hid, c] * w2[hid, d]
            pso = [
                psA.tile([P, D], F32, tag=f"eo{cc}", name=f"pso{cc}")
                for cc in range(CC)
            ]
            for g in range(2):
                w2t = w2pool.tile([P, HC // 2, D], BF16, tag="w2")
                nc.sync.dma_start(w2t, w2_ap[e, g])
                for hh in range(HC // 2):
                    hc = g * (HC // 2) + hh
                    for cc in range(CC):
                        nc.tensor.matmul(
                            pso[cc],
                            lhsT=h[:, hc, ds(cc * P, P)],
                            rhs=w2t[:, hh, :],
                            start=(hc == 0),
                            stop=(hc == HC - 1),
                        )
            for cc in range(CC):
                eo_sb = eopool.tile([P, D], F32R, tag="eo")
                nc.scalar.copy(eo_sb, pso[cc])
                nc.sync.dma_start(eod_ap[e, cc], eo_sb)

    ctxX.close()

    # ================= Phase B: combine =================
    with (
        tcx.tile_pool(name="cpool", bufs=2) as cpool,
        tcx.tile_pool(name="wmpool", bufs=2) as wmpool,
        tcx.tile_pool(name="eold", bufs=2) as eold,
        tcx.tile_pool(name="ostage", bufs=3) as ostage,
        tcx.tile_pool(name="psB", bufs=1, space="PSUM") as psB,
    ):
        for half in range(2):
            psO = [
                psB.tile([P, D], F32, tag=f"o{i}", name=f"psO{i}")
                for i in range(TC // 2)
            ]
            for e in range(E):
                wmb = wmpool.tile([P, NHALF], F32, tag="wmb")
                src = wmd_ap[e, ds(half * NHALF, NHALF)]
                src_b = bass.AP(
                    tensor=src.tensor, offset=src.offset, ap=[[0, P], *src.ap]
                )
                nc.gpsimd.dma_start(wmb, src_b)
                eot = eold.tile([P, CC, D], F32R, tag="eot")
                nc.sync.dma_start(eot, eod_ap[e].rearrange("c p d -> p c d"))
                C_e = cpool.tile([P, CC, NHALF], F32R, tag="C")
                for cc in range(CC):
                    nc.vector.tensor_mul(
                        C_e[:, cc, :], oh_T[:, cc, ds(half * NHALF, NHALF)], wmb
                    )
                for i in range(TC // 2):
                    for cc in range(CC):
                        nc.tensor.matmul(
                            psO[i],
                            lhsT=C_e[:, cc, ts(i, P)],
                            rhs=eot[:, cc, :],
                            start=(e == 0 and cc == 0),
                            stop=(e == E - 1 and cc == CC - 1),
                        )
            for i in range(TC // 2):
                o_sb = ostage.tile([P, D], F32, tag="osb")
                nc.scalar.copy(o_sb, psO[i])
                nc.sync.dma_start(out_t[half * (TC // 2) + i], o_sb)

    ctx.close()


_PROGRAM_CACHE = {}


def _get_program():
    if "nc" not in _PROGRAM_CACHE:
        _PROGRAM_CACHE["nc"] = build_program()
    return _PROGRAM_CACHE["nc"]


def _prep_weights(w1, w2):
    # w1s[e, g, p, hh, dc, hp] = w1[e, dc*128+p, (g*8+hh)*128+hp]
    w1s = np.ascontiguousarray(
        w1.reshape(E, DC, P, 2, HC // 2, P).transpose(0, 3, 2, 4, 1, 5)
    ).astype(BF16_NP)
    # w2s[e, g, p, hh, d] = w2[e, (g*8+hh)*128+p, d]
    w2s = np.ascontiguousarray(
        w2.reshape(E, 2, HC // 2, P, D).transpose(0, 1, 3, 2, 4)
    ).astype(BF16_NP)
    return w1s, w2s


def _run(x, w_gating, w1, w2, trace=False, **kwargs):
    from concourse.bass_utils import run_bass_kernel_spmd

    nc = _get_program()
    x = np.asarray(x, dtype=np.float32)
    wg = np.ascontiguousarray(np.asarray(w_gating, dtype=np.float32))
    w1s, w2s = _prep_weights(
        np.asarray(w1, dtype=np.float32), np.asarray(w2, dtype=np.float32)
    )
    in_maps = [
        {
            "x": np.ascontiguousarray(x[b]),
            "wg": wg,
            "w1s": w1s,
            "w2s": w2s,
        }
        for b in range(NCORES)
    ]
    res = run_bass_kernel_spmd(nc, in_maps, list(range(NCORES)), trace=trace, **kwargs)
    out = np.stack([res.results[b]["out"] for b in range(NCORES)], axis=0)
    lps = np.array(
        [np.float64(res.results[b]["lossp"].reshape(())) for b in range(NCORES)]
    )
    loss = np.float32(lps.sum() * E * LOSS_COEF / (float(N) * N * B))
    return (out, loss), res


def kernel(x, w_gating, w1, w2):
    (out, loss), _ = _run(x, w_gating, w1, w2, trace=False)
    return out, loss


if __name__ == "__main__":
    nc = _get_program()
    print("program built and compiled OK")


# revision 19
# speedup vs baseline: 1.2018x; 1.0192x over previous
"""MoE (threshold top-k routing, eval capacity) Trainium2 Bass kernel.

Strategy: data-parallel over the batch dim b (B=8 -> one batch element per
NeuronCore), full expert set computed locally on each core (no collectives).

Per-core program (N=2048 tokens, D=512, E=16, HID=2048, CAP=256):
  1. Gating logits via fp32 PE matmul (precision-critical: routing threshold
     margins are ~3e-5, so the gate path stays fp32 end-to-end).
  2. Softmax on ScalarE/VectorE; threshold top-k WITHOUT sorting via pairwise
     prob comparisons (expert e selected iff sum of probs strictly greater --
     ties broken by index -- is < 0.8).  Bit-matches jnp argsort semantics.
  3. Capacity: exclusive cumsum over tokens via the DVE scan instruction on an
     expert-major [16, 2048] layout; summed-slot quirk (pos_tok = sum over
     experts) reproduced exactly.
  4. Dispatch/combine as one-hot matmuls (handles duplicate-slot collisions by
     summation, exactly like the reference einsum).
  5. Expert FFN (gelu) with bf16 weights x float32r activations, fp32 PSUM.
"""

import sys

import numpy as np

sys.path.insert(0, "/opt/trn_rl_repo")

import ml_dtypes  # noqa: E402

import concourse.bass as bass  # noqa: E402
import concourse.mybir as mybir  # noqa: E402
import concourse.tile as tile  # noqa: E402
from concourse import bacc  # noqa: E402
from concourse.bass import ds, ts  # noqa: E402
from concourse.masks import make_identity  # noqa: E402

F32 = mybir.dt.float32
F32R = mybir.dt.float32r
BF16 = mybir.dt.bfloat16
I32 = mybir.dt.int32
AX = mybir.AxisListType
OP = mybir.AluOpType
AF = mybir.ActivationFunctionType

P = 128
B, N, D, E, HID, CAP = 8, 2048, 512, 16, 2048, 256
TC, DC, HC, CC = N // P, D // P, HID // P, CAP // P
NHALF = N // 2
THRESH = 0.8
LOSS_COEF = 0.01
NCORES = 8

BF16_NP = ml_dtypes.bfloat16


def _r(ap):
    """View an fp32 AP as float32r for 1-cycle/row PE matmuls."""
    return ap.bitcast(F32R)


def build_program():
    nc = bacc.Bacc(
        "TRN2",
        target_bir_lowering=False,
        debug=False,
        enable_asserts=False,
        num_devices=1,
    )

    x_ap = nc.dram_tensor("x", [N, D], F32, kind="ExternalInput").ap()
    wg_ap = nc.dram_tensor("wg", [D, E], F32, kind="ExternalInput").ap()
    # w1s[e, g, p, hh, dc, hp] = w1[e, dc*128+p, (g*8+hh)*128+hp]  (host-prepped)
    w1_ap = nc.dram_tensor("w1s", [E, 2, P, HC // 2, DC, P], BF16, kind="ExternalInput").ap()
    # w2s[e, g, p, hh, d] = w2[e, (g*8+hh)*128+p, d]  (host-prepped)
    w2_ap = nc.dram_tensor("w2s", [E, 2, P, HC // 2, D], BF16, kind="ExternalInput").ap()
    out_ap = nc.dram_tensor("out", [N, D], F32, kind="ExternalOutput").ap()
    lossp_ap = nc.dram_tensor("lossp", [1, 1], F32, kind="ExternalOutput").ap()
    # internal DRAM bounce buffers
    eod_ap = nc.dram_tensor("eod", [E, CC, P, D], F32R).ap()
    wmd_ap = nc.dram_tensor("wmd", [E, N], F32).ap()

    out_t = out_ap.rearrange("(t p) d -> t p d", p=P)

    with tile.TileContext(nc) as tcx:
        _emit(tcx, x_ap, wg_ap, w1_ap, w2_ap, out_t, lossp_ap, eod_ap, wmd_ap)

    nc.compile()
    return nc


def _emit(tcx, x_ap, wg_ap, w1_ap, w2_ap, out_t, lossp_ap, eod_ap, wmd_ap):
    from contextlib import ExitStack

    nc = tcx.nc
    ctx = ExitStack()
    const = ctx.enter_context(tcx.tile_pool(name="const", bufs=1))
    keep = ctx.enter_context(tcx.tile_pool(name="keep", bufs=1))
    ctxR = ExitStack()
    rp = ctxR.enter_context(tcx.tile_pool(name="routing", bufs=1))
    pw = ctxR.enter_context(tcx.tile_pool(name="pairwise", bufs=2))
    psT = ctxR.enter_context(tcx.tile_pool(name="psT", bufs=6, space="PSUM"))

    # ---------------- constants ----------------
    ident = const.tile([P, P], F32, name="ident")
    make_identity(nc, ident)
    ones16 = const.tile([E, 1], F32, name="ones16")
    nc.vector.memset(ones16, 1.0)
    iota_ci = const.tile([P, CAP], I32, name="iota_ci")
    nc.gpsimd.iota(iota_ci, pattern=[[1, CAP]], base=0, channel_multiplier=0)
    iota_c = const.tile([P, CAP], F32, name="iota_c")
    nc.vector.tensor_copy(iota_c, iota_ci)
    iota_pi = const.tile([P, CC], I32, name="iota_pi")
    nc.gpsimd.iota(iota_pi, pattern=[[P, CC]], base=0, channel_multiplier=1)
    iota_p = const.tile([P, CC], F32, name="iota_p")
    nc.vector.tensor_copy(iota_p, iota_pi)
    # tri[p, ep, e] = 1.0 where e > ep else 0.0 (tie-break mask)
    tri = const.tile([P, E, E], F32, name="tri")
    nc.gpsimd.memset(tri, 1.0)
    nc.gpsimd.affine_select(
        out=tri, in_=tri, pattern=[[-1, E], [1, E]], base=0,
        channel_multiplier=0, compare_op=OP.is_gt, fill=0.0,
    )

    # ---------------- load x, wg ----------------
    x_sb = rp.tile([P, TC, D], F32, name="x_sb")
    nc.sync.dma_start(x_sb, x_ap.rearrange("(t p) d -> p t d", p=P))
    wg_sb = rp.tile([P, DC, E], F32, name="wg_sb")
    nc.sync.dma_start(wg_sb, wg_ap.rearrange("(c p) e -> p c e", p=P))

    # ---------------- transpose x -> xT [d-part, dc, t] ----------------
    with tcx.tile_pool(name="xt", bufs=1) as xtp:
        xT = xtp.tile([P, DC, N], F32, name="xT")
        for t in range(TC):
            for dc in range(DC):
                ps = psT.tile([P, P], F32, tag="ps")
                nc.tensor.transpose(ps, x_sb[:, t, ds(dc * P, P)], ident)
                nc.scalar.copy(xT[:, dc, ds(t * P, P)], ps)

        # ---------------- gating logits [t-part, tc, e] (full fp32) ------
        lg = rp.tile([P, TC, E], F32, name="lg")
        for t in range(TC):
            ps = psT.tile([P, E], F32, tag="ps")
            for dc in range(DC):
                nc.tensor.matmul(
                    ps, lhsT=xT[:, dc, ts(t, P)], rhs=wg_sb[:, dc, :],
                    start=(dc == 0), stop=(dc == DC - 1),
                )
            nc.scalar.copy(lg[:, t, :], ps)

    # persistent f32r copy of x for the dispatch matmuls (after xT freed)
    ctxX = ExitStack()
    xrp = ctxX.enter_context(tcx.tile_pool(name="xrp", bufs=1, side="right"))
    x_r = xrp.tile([P, TC, D], BF16, name="x_r")
    nc.vector.tensor_copy(x_r, x_sb)

    # ---------------- softmax (fp32) ----------------
    rmax = rp.tile([P, TC], F32, name="rmax")
    nc.vector.reduce_max(rmax, lg, axis=AX.X)
    sh = rp.tile([P, TC, E], F32, name="sh")
    nc.vector.tensor_sub(sh, lg, rmax[:, :, None].to_broadcast([P, TC, E]))
    u = rp.tile([P, TC, E], F32, name="u")
    nc.scalar.activation(u, sh, AF.Exp)
    usum = rp.tile([P, TC], F32, name="usum")
    nc.vector.reduce_sum(usum, u, axis=AX.X)
    rin = rp.tile([P, TC], F32, name="rin")
    nc.vector.reciprocal(rin, usum)
    p_sb = rp.tile([P, TC, E], F32, name="p_sb")
    nc.vector.tensor_mul(p_sb, u, rin[:, :, None].to_broadcast([P, TC, E]))

    # ------------- threshold top-k via pairwise comparisons -------------
    # S[t, e] = sum_{e'} p[t,e'] * [ p_e' > p_e  or (p_e' == p_e and e' < e) ]
    # mask[t, e] = S[t, e] < THRESH      (== reference argsort/cumsum mask)
    S = rp.tile([P, TC, E], F32, name="S")
    nc.vector.memset(S, 0.0)
    for ep in range(E):
        eng = nc.vector
        acc = S
        sfx = ""
        pb = p_sb[:, :, ep : ep + 1].to_broadcast([P, TC, E])
        gt = pw.tile([P, TC, E], F32, tag="gt" + sfx)
        eng.tensor_tensor(gt, pb, p_sb, OP.is_gt)
        eq = pw.tile([P, TC, E], F32, tag="eq" + sfx)
        eng.tensor_tensor(eq, pb, p_sb, OP.is_equal)
        # ties only count for columns e > ep
        m = pw.tile([P, TC, E], F32, tag="m" + sfx)
        eng.tensor_mul(m, eq, tri[:, ep : ep + 1, :].to_broadcast([P, TC, E]))
        eng.tensor_add(m, m, gt)
        t2 = pw.tile([P, TC, E], F32, tag="t2" + sfx)
        eng.tensor_mul(t2, m, pb)
        eng.tensor_add(acc, acc, t2)

    mask = rp.tile([P, TC, E], F32, name="mask")
    nc.vector.tensor_single_scalar(mask, S, THRESH, OP.is_lt)
    selp = rp.tile([P, TC, E], F32, name="selp")
    nc.vector.tensor_mul(selp, p_sb, mask)
    wsum = rp.tile([P, TC], F32, name="wsum")
    nc.vector.reduce_sum(wsum, selp, axis=AX.X)
    winv = rp.tile([P, TC], F32, name="winv")
    nc.vector.reciprocal(winv, wsum)
    wts = rp.tile([P, TC, E], F32, name="wts")
    nc.vector.tensor_mul(wts, selp, winv[:, :, None].to_broadcast([P, TC, E]))

    # ------------- transpose mask/wts/p to expert-major [E, N] -------------
    mask_T = rp.tile([E, N], F32, name="mask_T")
    wts_T = rp.tile([E, N], F32, name="wts_T")
    p_T = rp.tile([E, N], F32, name="p_T")
    for t in range(TC):
        for src, dst in ((mask, mask_T), (wts, wts_T), (p_sb, p_T)):
            ps = psT.tile([E, P], F32, tag="ps")
            nc.tensor.transpose(ps, src[:, t, :], ident)
            nc.scalar.copy(dst[:, ds(t * P, P)], ps)

    # ------------- capacity (exclusive cumsum over tokens) -------------
    cum = rp.tile([E, N], F32, name="cum")
    nc.vector.tensor_tensor_scan(cum, mask_T, mask_T, 0.0, OP.add, OP.bypass)
    pos = rp.tile([E, N], F32, name="pos")
    nc.vector.tensor_sub(pos, cum, mask_T)
    mask2_T = rp.tile([E, N], F32, name="mask2_T")
    nc.vector.scalar_tensor_tensor(
        mask2_T, in0=pos, scalar=float(CAP), in1=mask_T, op0=OP.is_lt, op1=OP.mult
    )
    pos2 = rp.tile([E, N], F32, name="pos2")
    nc.vector.tensor_mul(pos2, pos, mask2_T)
    wm_T = rp.tile([E, N], F32, name="wm_T")
    nc.vector.tensor_mul(wm_T, wts_T, mask2_T)
    nc.sync.dma_start(wmd_ap, wm_T)

    # ------------- aux loss partial: sum_e mean_t(p) * mean_t(mask2) -----
    proxs = rp.tile([E, 1], F32, name="proxs")
    nc.vector.reduce_sum(proxs, p_T, axis=AX.X)
    denss = rp.tile([E, 1], F32, name="denss")
    nc.vector.reduce_sum(denss, mask2_T, axis=AX.X)
    prod = rp.tile([E, 1], F32, name="prod")
    nc.vector.tensor_mul(prod, proxs, denss)
    psl = psT.tile([1, 1], F32, tag="ps")
    nc.tensor.matmul(psl, lhsT=prod, rhs=ones16, start=True, stop=True)
    lp_sb = rp.tile([1, 1], F32, name="lp_sb")
    nc.scalar.copy(lp_sb, psl)
    nc.sync.dma_start(lossp_ap, lp_sb)

    # ------------- summed slot index s(t) = sum_e pos2  -------------
    # expert-major broadcast copy: s_bcast[c-part, t]
    s_row = rp.tile([1, N], F32, name="s_row")
    for q in range(N // 512):
        ps = psT.tile([1, 512], F32, tag="ps")
        nc.tensor.matmul(
            ps, lhsT=ones16, rhs=pos2[:, ds(q * 512, 512)], start=True, stop=True
        )
        nc.scalar.copy(s_row[:, ds(q * 512, 512)], ps)
    ones1 = const.tile([1, P], F32, name="ones1")
    nc.vector.memset(ones1, 1.0)
    s_bcast = rp.tile([P, N], F32, name="s_bcast")
    for q in range(N // 512):
        ps = psT.tile([P, 512], F32, tag="ps")
        nc.tensor.matmul(
            ps, lhsT=ones1, rhs=s_row[:, ds(q * 512, 512)], start=True, stop=True
        )
        nc.scalar.copy(s_bcast[:, ds(q * 512, 512)], ps)

    # token-major mask2 and s: transpose back
    mask2_tok = keep.tile([P, TC, E], F32, name="mask2_tok")
    pos2_tok = rp.tile([P, TC, E], F32, name="pos2_tok")
    for t in range(TC):
        for src, dst in ((mask2_T, mask2_tok), (pos2, pos2_tok)):
            ps = psT.tile([P, E], F32, tag="ps")
            nc.tensor.transpose(ps, src[:, ds(t * P, P)], ident[:E, :E])
            nc.scalar.copy(dst[:, t, :], ps)
    s_tok = rp.tile([P, TC], F32, name="s_tok")
    nc.vector.reduce_sum(s_tok, pos2_tok, axis=AX.X)

    # ------------- one-hot slot structures -------------
    oh_tok = keep.tile([P, TC, CAP], BF16, name="oh_tok")  # [t, c] = (s(t)==c)
    nc.vector.tensor_tensor(
        oh_tok,
        s_tok[:, :, None].to_broadcast([P, TC, CAP]),
        iota_c[:, None, :].to_broadcast([P, TC, CAP]),
        OP.is_equal,
    )
    oh_T = keep.tile([P, CC, N], BF16, name="oh_T")  # [c, t] = (s(t)==c)
    for cc in range(CC):
        nc.vector.tensor_tensor(
            oh_T[:, cc, :],
            s_bcast,
            iota_p[:, cc : cc + 1].to_broadcast([P, N]),
            OP.is_equal,
        )

    ctxR.close()

    # ================= Phase A: dispatch + expert FFN =================
    with (
        tcx.tile_pool(name="mpool", bufs=3) as mpool,
        tcx.tile_pool(name="w1pool", bufs=2) as w1pool,
        tcx.tile_pool(name="w2pool", bufs=2) as w2pool,
        tcx.tile_pool(name="eipool", bufs=2) as eipool,
        tcx.tile_pool(name="hpool", bufs=2) as hpool,
        tcx.tile_pool(name="eopool", bufs=3) as eopool,
        tcx.tile_pool(name="psA", bufs=1, space="PSUM") as psA,
        tcx.tile_pool(name="psH", bufs=2, space="PSUM") as psH,
    ):
        for e in range(E):
            # M_e[t, c] = oh[t, c] * mask2[t, e]   (bf16, exact 0/1 weights)
            M_e = mpool.tile([P, TC, CAP], BF16, tag="M")
            for t in range(TC):
                nc.vector.tensor_scalar_mul(
                    M_e[:, t, :], oh_tok[:, t, :], mask2_tok[:, t, e : e + 1]
                )
            # EI_T[d, c] = sum_t x[t, d] * M_e[t, c]
            ei = eipool.tile([P, DC, CAP], BF16, tag="ei")
            for dc in range(DC):
                pse = psA.tile([P, CAP], F32, tag=f"ei{dc}")
                for t in range(TC):
                    nc.tensor.matmul(
                        pse,
                        lhsT=x_r[:, t, ds(dc * P, P)],
                        rhs=M_e[:, t, :],
                        start=(t == 0),
                        stop=(t == TC - 1),
                    )
                nc.scalar.copy(ei[:, dc, :], pse)
            # h[hid, c] = gelu(sum_d w1[d, hid] * EI_T[d, c])
            h = hpool.tile([P, HC, CAP], BF16, tag="h")
            for g in range(2):
                w1t = w1pool.tile([P, HC // 2, DC, P], BF16, tag="w1")
                nc.sync.dma_start(w1t, w1_ap[e, g])
                for hh in range(HC // 2):
                    psh = psH.tile([P, CAP], F32, tag="h")
                    for dc in range(DC):
                        nc.tensor.matmul(
                            psh,
                            lhsT=w1t[:, hh, dc, :],
                            rhs=ei[:, dc, :],
                            start=(dc == 0),
                            stop=(dc == DC - 1),
                        )
                    nc.scalar.activation(h[:, g * (HC // 2) + hh, :], psh, AF.Gelu)
            # EO[c, d] = sum_hid h[hid, c] * w2[hid, d]
            pso = [
                psA.tile([P, D], F32, tag=f"eo{cc}", name=f"pso{cc}")
                for cc in range(CC)
            ]
            for g in range(2):
                w2t = w2pool.tile([P, HC // 2, D], BF16, tag="w2")
                nc.sync.dma_start(w2t, w2_ap[e, g])
                for hh in range(HC // 2):
                    hc = g * (HC // 2) + hh
                    for cc in range(CC):
                        nc.tensor.matmul(
                            pso[cc],
                            lhsT=h[:, hc, ds(cc * P, P)],
                            rhs=w2t[:, hh, :],
                            start=(hc == 0),
                            stop=(hc == HC - 1),
                        )
            for cc in range(CC):
                eo_sb = eopool.tile([P, D], F32R, tag="eo")
                nc.scalar.copy(eo_sb, pso[cc])
                nc.sync.dma_start(eod_ap[e, cc], eo_sb)

    ctxX.close()

    # ================= Phase B: combine =================
    with (
        tcx.tile_pool(name="cpool", bufs=3) as cpool,
        tcx.tile_pool(name="wmpool", bufs=3) as wmpool,
        tcx.tile_pool(name="eold", bufs=3) as eold,
        tcx.tile_pool(name="ostage", bufs=3) as ostage,
        tcx.tile_pool(name="psB", bufs=1, space="PSUM") as psB,
    ):
        for half in range(2):
            psO = [
                psB.tile([P, D], F32, tag=f"o{i}", name=f"psO{i}")
                for i in range(TC // 2)
            ]
            for e in range(E):
                wmb = wmpool.tile([P, NHALF], F32, tag="wmb")
                src = wmd_ap[e, ds(half * NHALF, NHALF)]
                src_b = bass.AP(
                    tensor=src.tensor, offset=src.offset, ap=[[0, P], *src.ap]
                )
                nc.gpsimd.dma_start(wmb, src_b)
                eot = eold.tile([P, CC, D], F32R, tag="eot")
                nc.sync.dma_start(eot, eod_ap[e].rearrange("c p d -> p c d"))
                C_e = cpool.tile([P, CC, NHALF], F32R, tag="C")
                for cc in range(CC):
                    nc.vector.tensor_mul(
                        C_e[:, cc, :], oh_T[:, cc, ds(half * NHALF, NHALF)], wmb
                    )
                for i in range(TC // 2):
                    for cc in range(CC):
                        nc.tensor.matmul(
                            psO[i],
                            lhsT=C_e[:, cc, ts(i, P)],
                            rhs=eot[:, cc, :],
                            start=(e == 0 and cc == 0),
                            stop=(e == E - 1 and cc == CC - 1),
                        )
            for i in range(TC // 2):
                o_sb = ostage.tile([P, D], F32, tag="osb")
                nc.scalar.copy(o_sb, psO[i])
                nc.sync.dma_start(out_t[half * (TC // 2) + i], o_sb)

    ctx.close()


_PROGRAM_CACHE = {}


def _get_program():
    if "nc" not in _PROGRAM_CACHE:
        _PROGRAM_CACHE["nc"] = build_program()
    return _PROGRAM_CACHE["nc"]


def _prep_weights(w1, w2):
    # w1s[e, g, p, hh, dc, hp] = w1[e, dc*128+p, (g*8+hh)*128+hp]
    w1s = np.ascontiguousarray(
        w1.reshape(E, DC, P, 2, HC // 2, P).transpose(0, 3, 2, 4, 1, 5)
    ).astype(BF16_NP)
    # w2s[e, g, p, hh, d] = w2[e, (g*8+hh)*128+p, d]
    w2s = np.ascontiguousarray(
        w2.reshape(E, 2, HC // 2, P, D).transpose(0, 1, 3, 2, 4)
    ).astype(BF16_NP)
    return w1s, w2s


def _run(x, w_gating, w1, w2, trace=False, **kwargs):
    from concourse.bass_utils import run_bass_kernel_spmd

    nc = _get_program()
    x = np.asarray(x, dtype=np.float32)
    wg = np.ascontiguousarray(np.asarray(w_gating, dtype=np.float32))
    w1s, w2s = _prep_weights(
        np.asarray(w1, dtype=np.float32), np.asarray(w2, dtype=np.float32)
    )
    in_maps = [
        {
            "x": np.ascontiguousarray(x[b]),
            "wg": wg,
            "w1s": w1s,
            "w2s": w2s,
        }
        for b in range(NCORES)
    ]
    res = run_bass_kernel_spmd(nc, in_maps, list(range(NCORES)), trace=trace, **kwargs)
    out = np.stack([res.results[b]["out"] for b in range(NCORES)], axis=0)
    lps = np.array(
        [np.float64(res.results[b]["lossp"].reshape(())) for b in range(NCORES)]
    )
    loss = np.float32(lps.sum() * E * LOSS_COEF / (float(N) * N * B))
    return (out, loss), res


def kernel(x, w_gating, w1, w2):
    (out, loss), _ = _run(x, w_gating, w1, w2, trace=False)
    return out, loss


if __name__ == "__main__":
    nc = _get_program()
    print("program built and compiled OK")


# revision 20
# speedup vs baseline: 1.2464x; 1.0371x over previous
"""MoE (threshold top-k routing, eval capacity) Trainium2 Bass kernel.

Strategy: data-parallel over the batch dim b (B=8 -> one batch element per
NeuronCore), full expert set computed locally on each core (no collectives).

Per-core program (N=2048 tokens, D=512, E=16, HID=2048, CAP=256):
  1. Gating logits via fp32 PE matmul (precision-critical: routing threshold
     margins are ~3e-5, so the gate path stays fp32 end-to-end).
  2. Softmax on ScalarE/VectorE; threshold top-k WITHOUT sorting via pairwise
     prob comparisons (expert e selected iff sum of probs strictly greater --
     ties broken by index -- is < 0.8).  Bit-matches jnp argsort semantics.
  3. Capacity: exclusive cumsum over tokens via the DVE scan instruction on an
     expert-major [16, 2048] layout; summed-slot quirk (pos_tok = sum over
     experts) reproduced exactly.
  4. Dispatch/combine as one-hot matmuls (handles duplicate-slot collisions by
     summation, exactly like the reference einsum).
  5. Expert FFN (gelu) with bf16 weights x float32r activations, fp32 PSUM.
"""

import sys

import numpy as np

sys.path.insert(0, "/opt/trn_rl_repo")

import ml_dtypes  # noqa: E402

import concourse.bass as bass  # noqa: E402
import concourse.mybir as mybir  # noqa: E402
import concourse.tile as tile  # noqa: E402
from concourse import bacc  # noqa: E402
from concourse.bass import ds, ts  # noqa: E402
from concourse.masks import make_identity  # noqa: E402

F32 = mybir.dt.float32
F32R = mybir.dt.float32r
BF16 = mybir.dt.bfloat16
I32 = mybir.dt.int32
AX = mybir.AxisListType
OP = mybir.AluOpType
AF = mybir.ActivationFunctionType

P = 128
B, N, D, E, HID, CAP = 8, 2048, 512, 16, 2048, 256
TC, DC, HC, CC = N // P, D // P, HID // P, CAP // P
NHALF = N // 2
THRESH = 0.8
LOSS_COEF = 0.01
NCORES = 8

BF16_NP = ml_dtypes.bfloat16


def _r(ap):
    """View an fp32 AP as float32r for 1-cycle/row PE matmuls."""
    return ap.bitcast(F32R)


def build_program():
    nc = bacc.Bacc(
        "TRN2",
        target_bir_lowering=False,
        debug=False,
        enable_asserts=False,
        num_devices=1,
    )

    x_ap = nc.dram_tensor("x", [N, D], F32, kind="ExternalInput").ap()
    wg_ap = nc.dram_tensor("wg", [D, E], F32, kind="ExternalInput").ap()
    # w1s[e, g, p, hh, dc, hp] = w1[e, dc*128+p, (g*8+hh)*128+hp]  (host-prepped)
    w1_ap = nc.dram_tensor("w1s", [E, 2, P, HC // 2, DC, P], BF16, kind="ExternalInput").ap()
    # w2s[e, g, p, hh, d] = w2[e, (g*8+hh)*128+p, d]  (host-prepped)
    w2_ap = nc.dram_tensor("w2s", [E, 2, P, HC // 2, D], BF16, kind="ExternalInput").ap()
    out_ap = nc.dram_tensor("out", [N, D], F32, kind="ExternalOutput").ap()
    lossp_ap = nc.dram_tensor("lossp", [1, 1], F32, kind="ExternalOutput").ap()
    # internal DRAM bounce buffers
    wmd_ap = nc.dram_tensor("wmd", [E, N], F32).ap()

    out_t = out_ap.rearrange("(t p) d -> t p d", p=P)

    with tile.TileContext(nc) as tcx:
        _emit(tcx, x_ap, wg_ap, w1_ap, w2_ap, out_t, lossp_ap, wmd_ap)

    nc.compile()
    return nc


def _emit(tcx, x_ap, wg_ap, w1_ap, w2_ap, out_t, lossp_ap, wmd_ap):
    from contextlib import ExitStack

    nc = tcx.nc
    ctx = ExitStack()
    const = ctx.enter_context(tcx.tile_pool(name="const", bufs=1))
    keep = ctx.enter_context(tcx.tile_pool(name="keep", bufs=1))
    ctxR = ExitStack()
    rp = ctxR.enter_context(tcx.tile_pool(name="routing", bufs=1))
    pw = ctxR.enter_context(tcx.tile_pool(name="pairwise", bufs=2))
    psT = ctxR.enter_context(tcx.tile_pool(name="psT", bufs=6, space="PSUM"))

    # ---------------- constants ----------------
    ident = const.tile([P, P], F32, name="ident")
    make_identity(nc, ident)
    ones16 = const.tile([E, 1], F32, name="ones16")
    nc.vector.memset(ones16, 1.0)
    iota_ci = const.tile([P, CAP], I32, name="iota_ci")
    nc.gpsimd.iota(iota_ci, pattern=[[1, CAP]], base=0, channel_multiplier=0)
    iota_c = const.tile([P, CAP], F32, name="iota_c")
    nc.vector.tensor_copy(iota_c, iota_ci)
    iota_pi = const.tile([P, CC], I32, name="iota_pi")
    nc.gpsimd.iota(iota_pi, pattern=[[P, CC]], base=0, channel_multiplier=1)
    iota_p = const.tile([P, CC], F32, name="iota_p")
    nc.vector.tensor_copy(iota_p, iota_pi)
    # tri[p, ep, e] = 1.0 where e > ep else 0.0 (tie-break mask)
    tri = const.tile([P, E, E], F32, name="tri")
    nc.gpsimd.memset(tri, 1.0)
    nc.gpsimd.affine_select(
        out=tri, in_=tri, pattern=[[-1, E], [1, E]], base=0,
        channel_multiplier=0, compare_op=OP.is_gt, fill=0.0,
    )

    # ---------------- load x, wg ----------------
    x_sb = rp.tile([P, TC, D], F32, name="x_sb")
    nc.sync.dma_start(x_sb, x_ap.rearrange("(t p) d -> p t d", p=P))
    wg_sb = rp.tile([P, DC, E], F32, name="wg_sb")
    nc.sync.dma_start(wg_sb, wg_ap.rearrange("(c p) e -> p c e", p=P))

    # ---------------- transpose x -> xT [d-part, dc, t] ----------------
    with tcx.tile_pool(name="xt", bufs=1) as xtp:
        xT = xtp.tile([P, DC, N], F32, name="xT")
        for t in range(TC):
            for dc in range(DC):
                ps = psT.tile([P, P], F32, tag="ps")
                nc.tensor.transpose(ps, x_sb[:, t, ds(dc * P, P)], ident)
                nc.scalar.copy(xT[:, dc, ds(t * P, P)], ps)

        # ---------------- gating logits [t-part, tc, e] (full fp32) ------
        lg = rp.tile([P, TC, E], F32, name="lg")
        for t in range(TC):
            ps = psT.tile([P, E], F32, tag="ps")
            for dc in range(DC):
                nc.tensor.matmul(
                    ps, lhsT=xT[:, dc, ts(t, P)], rhs=wg_sb[:, dc, :],
                    start=(dc == 0), stop=(dc == DC - 1),
                )
            nc.scalar.copy(lg[:, t, :], ps)

    # persistent f32r copy of x for the dispatch matmuls (after xT freed)
    ctxX = ExitStack()
    xrp = ctxX.enter_context(tcx.tile_pool(name="xrp", bufs=1, side="right"))
    x_r = xrp.tile([P, TC, D], BF16, name="x_r")
    nc.vector.tensor_copy(x_r, x_sb)

    # ---------------- softmax (fp32) ----------------
    rmax = rp.tile([P, TC], F32, name="rmax")
    nc.vector.reduce_max(rmax, lg, axis=AX.X)
    sh = rp.tile([P, TC, E], F32, name="sh")
    nc.vector.tensor_sub(sh, lg, rmax[:, :, None].to_broadcast([P, TC, E]))
    u = rp.tile([P, TC, E], F32, name="u")
    nc.scalar.activation(u, sh, AF.Exp)
    usum = rp.tile([P, TC], F32, name="usum")
    nc.vector.reduce_sum(usum, u, axis=AX.X)
    rin = rp.tile([P, TC], F32, name="rin")
    nc.vector.reciprocal(rin, usum)
    p_sb = rp.tile([P, TC, E], F32, name="p_sb")
    nc.vector.tensor_mul(p_sb, u, rin[:, :, None].to_broadcast([P, TC, E]))

    # ------------- threshold top-k via pairwise comparisons -------------
    # S[t, e] = sum_{e'} p[t,e'] * [ p_e' > p_e  or (p_e' == p_e and e' < e) ]
    # mask[t, e] = S[t, e] < THRESH      (== reference argsort/cumsum mask)
    S = rp.tile([P, TC, E], F32, name="S")
    nc.vector.memset(S, 0.0)
    for ep in range(E):
        eng = nc.vector
        acc = S
        sfx = ""
        pb = p_sb[:, :, ep : ep + 1].to_broadcast([P, TC, E])
        gt = pw.tile([P, TC, E], F32, tag="gt" + sfx)
        eng.tensor_tensor(gt, pb, p_sb, OP.is_gt)
        eq = pw.tile([P, TC, E], F32, tag="eq" + sfx)
        eng.tensor_tensor(eq, pb, p_sb, OP.is_equal)
        # ties only count for columns e > ep
        m = pw.tile([P, TC, E], F32, tag="m" + sfx)
        eng.tensor_mul(m, eq, tri[:, ep : ep + 1, :].to_broadcast([P, TC, E]))
        eng.tensor_add(m, m, gt)
        t2 = pw.tile([P, TC, E], F32, tag="t2" + sfx)
        eng.tensor_mul(t2, m, pb)
        eng.tensor_add(acc, acc, t2)

    mask = rp.tile([P, TC, E], F32, name="mask")
    nc.vector.tensor_single_scalar(mask, S, THRESH, OP.is_lt)
    selp = rp.tile([P, TC, E], F32, name="selp")
    nc.vector.tensor_mul(selp, p_sb, mask)
    wsum = rp.tile([P, TC], F32, name="wsum")
    nc.vector.reduce_sum(wsum, selp, axis=AX.X)
    winv = rp.tile([P, TC], F32, name="winv")
    nc.vector.reciprocal(winv, wsum)
    wts = rp.tile([P, TC, E], F32, name="wts")
    nc.vector.tensor_mul(wts, selp, winv[:, :, None].to_broadcast([P, TC, E]))

    # ------------- transpose mask/wts/p to expert-major [E, N] -------------
    mask_T = rp.tile([E, N], F32, name="mask_T")
    wts_T = rp.tile([E, N], F32, name="wts_T")
    p_T = rp.tile([E, N], F32, name="p_T")
    for t in range(TC):
        for src, dst in ((mask, mask_T), (wts, wts_T), (p_sb, p_T)):
            ps = psT.tile([E, P], F32, tag="ps")
            nc.tensor.transpose(ps, src[:, t, :], ident)
            nc.scalar.copy(dst[:, ds(t * P, P)], ps)

    # ------------- capacity (exclusive cumsum over tokens) -------------
    cum = rp.tile([E, N], F32, name="cum")
    nc.vector.tensor_tensor_scan(cum, mask_T, mask_T, 0.0, OP.add, OP.bypass)
    pos = rp.tile([E, N], F32, name="pos")
    nc.vector.tensor_sub(pos, cum, mask_T)
    mask2_T = rp.tile([E, N], F32, name="mask2_T")
    nc.vector.scalar_tensor_tensor(
        mask2_T, in0=pos, scalar=float(CAP), in1=mask_T, op0=OP.is_lt, op1=OP.mult
    )
    pos2 = rp.tile([E, N], F32, name="pos2")
    nc.vector.tensor_mul(pos2, pos, mask2_T)
    wm_T = rp.tile([E, N], F32, name="wm_T")
    nc.vector.tensor_mul(wm_T, wts_T, mask2_T)
    nc.sync.dma_start(wmd_ap, wm_T)

    # ------------- aux loss partial: sum_e mean_t(p) * mean_t(mask2) -----
    proxs = rp.tile([E, 1], F32, name="proxs")
    nc.vector.reduce_sum(proxs, p_T, axis=AX.X)
    denss = rp.tile([E, 1], F32, name="denss")
    nc.vector.reduce_sum(denss, mask2_T, axis=AX.X)
    prod = rp.tile([E, 1], F32, name="prod")
    nc.vector.tensor_mul(prod, proxs, denss)
    psl = psT.tile([1, 1], F32, tag="ps")
    nc.tensor.matmul(psl, lhsT=prod, rhs=ones16, start=True, stop=True)
    lp_sb = rp.tile([1, 1], F32, name="lp_sb")
    nc.scalar.copy(lp_sb, psl)
    nc.sync.dma_start(lossp_ap, lp_sb)

    # ------------- summed slot index s(t) = sum_e pos2  -------------
    # expert-major broadcast copy: s_bcast[c-part, t]
    s_row = rp.tile([1, N], F32, name="s_row")
    for q in range(N // 512):
        ps = psT.tile([1, 512], F32, tag="ps")
        nc.tensor.matmul(
            ps, lhsT=ones16, rhs=pos2[:, ds(q * 512, 512)], start=True, stop=True
        )
        nc.scalar.copy(s_row[:, ds(q * 512, 512)], ps)
    ones1 = const.tile([1, P], F32, name="ones1")
    nc.vector.memset(ones1, 1.0)
    s_bcast = rp.tile([P, N], F32, name="s_bcast")
    for q in range(N // 512):
        ps = psT.tile([P, 512], F32, tag="ps")
        nc.tensor.matmul(
            ps, lhsT=ones1, rhs=s_row[:, ds(q * 512, 512)], start=True, stop=True
        )
        nc.scalar.copy(s_bcast[:, ds(q * 512, 512)], ps)

    # token-major mask2 and s: transpose back
    mask2_tok = keep.tile([P, TC, E], F32, name="mask2_tok")
    pos2_tok = rp.tile([P, TC, E], F32, name="pos2_tok")
    for t in range(TC):
        for src, dst in ((mask2_T, mask2_tok), (pos2, pos2_tok)):
            ps = psT.tile([P, E], F32, tag="ps")
            nc.tensor.transpose(ps, src[:, ds(t * P, P)], ident[:E, :E])
            nc.scalar.copy(dst[:, t, :], ps)
    s_tok = rp.tile([P, TC], F32, name="s_tok")
    nc.vector.reduce_sum(s_tok, pos2_tok, axis=AX.X)

    # ------------- one-hot slot structures -------------
    oh_tok = keep.tile([P, TC, CAP], BF16, name="oh_tok")  # [t, c] = (s(t)==c)
    nc.vector.tensor_tensor(
        oh_tok,
        s_tok[:, :, None].to_broadcast([P, TC, CAP]),
        iota_c[:, None, :].to_broadcast([P, TC, CAP]),
        OP.is_equal,
    )
    oh_T = keep.tile([P, CC, N], BF16, name="oh_T")  # [c, t] = (s(t)==c)
    for cc in range(CC):
        nc.vector.tensor_tensor(
            oh_T[:, cc, :],
            s_bcast,
            iota_p[:, cc : cc + 1].to_broadcast([P, N]),
            OP.is_equal,
        )

    ctxR.close()

    eoallp = ctx.enter_context(tcx.tile_pool(name="eoall", bufs=1))
    eo_all = eoallp.tile([P, E, CC, D], F32R, name="eo_all")

    # ================= Phase A: dispatch + expert FFN =================
    with (
        tcx.tile_pool(name="mpool", bufs=2) as mpool,
        tcx.tile_pool(name="w1pool", bufs=2) as w1pool,
        tcx.tile_pool(name="w2pool", bufs=2) as w2pool,
        tcx.tile_pool(name="eipool", bufs=2) as eipool,
        tcx.tile_pool(name="hpool", bufs=2) as hpool,
        tcx.tile_pool(name="psA", bufs=1, space="PSUM") as psA,
        tcx.tile_pool(name="psH", bufs=2, space="PSUM") as psH,
    ):
        for e in range(E):
            # M_e[t, c] = oh[t, c] * mask2[t, e]   (bf16, exact 0/1 weights)
            M_e = mpool.tile([P, TC, CAP], BF16, tag="M")
            for t in range(TC):
                nc.vector.tensor_scalar_mul(
                    M_e[:, t, :], oh_tok[:, t, :], mask2_tok[:, t, e : e + 1]
                )
            # EI_T[d, c] = sum_t x[t, d] * M_e[t, c]
            ei = eipool.tile([P, DC, CAP], BF16, tag="ei")
            for dc in range(DC):
                pse = psA.tile([P, CAP], F32, tag=f"ei{dc}")
                for t in range(TC):
                    nc.tensor.matmul(
                        pse,
                        lhsT=x_r[:, t, ds(dc * P, P)],
                        rhs=M_e[:, t, :],
                        start=(t == 0),
                        stop=(t == TC - 1),
                    )
                nc.scalar.copy(ei[:, dc, :], pse)
            # h[hid, c] = gelu(sum_d w1[d, hid] * EI_T[d, c])
            h = hpool.tile([P, HC, CAP], BF16, tag="h")
            for g in range(2):
                w1t = w1pool.tile([P, HC // 2, DC, P], BF16, tag="w1")
                nc.sync.dma_start(w1t, w1_ap[e, g])
                for hh in range(HC // 2):
                    psh = psH.tile([P, CAP], F32, tag="h")
                    for dc in range(DC):
                        nc.tensor.matmul(
                            psh,
                            lhsT=w1t[:, hh, dc, :],
                            rhs=ei[:, dc, :],
                            start=(dc == 0),
                            stop=(dc == DC - 1),
                        )
                    nc.scalar.activation(h[:, g * (HC // 2) + hh, :], psh, AF.Gelu)
            # EO[c, d] = sum_hid h[hid, c] * w2[hid, d]
            pso = [
                psA.tile([P, D], F32, tag=f"eo{cc}", name=f"pso{cc}")
                for cc in range(CC)
            ]
            for g in range(2):
                w2t = w2pool.tile([P, HC // 2, D], BF16, tag="w2")
                nc.sync.dma_start(w2t, w2_ap[e, g])
                for hh in range(HC // 2):
                    hc = g * (HC // 2) + hh
                    for cc in range(CC):
                        nc.tensor.matmul(
                            pso[cc],
                            lhsT=h[:, hc, ds(cc * P, P)],
                            rhs=w2t[:, hh, :],
                            start=(hc == 0),
                            stop=(hc == HC - 1),
                        )
            for cc in range(CC):
                nc.scalar.copy(eo_all[:, e, cc, :], pso[cc])

    ctxX.close()

    # ================= Phase B: combine =================
    with (
        tcx.tile_pool(name="cpool", bufs=3) as cpool,
        tcx.tile_pool(name="wmpool", bufs=3) as wmpool,
        tcx.tile_pool(name="ostage", bufs=3) as ostage,
        tcx.tile_pool(name="psB", bufs=1, space="PSUM") as psB,
    ):
        for half in range(2):
            psO = [
                psB.tile([P, D], F32, tag=f"o{i}", name=f"psO{i}")
                for i in range(TC // 2)
            ]
            for e in range(E):
                wmb = wmpool.tile([P, NHALF], F32, tag="wmb")
                src = wmd_ap[e, ds(half * NHALF, NHALF)]
                src_b = bass.AP(
                    tensor=src.tensor, offset=src.offset, ap=[[0, P], *src.ap]
                )
                nc.gpsimd.dma_start(wmb, src_b)
                C_e = cpool.tile([P, CC, NHALF], F32R, tag="C")
                for cc in range(CC):
                    nc.vector.tensor_mul(
                        C_e[:, cc, :], oh_T[:, cc, ds(half * NHALF, NHALF)], wmb
                    )
                for i in range(TC // 2):
                    for cc in range(CC):
                        nc.tensor.matmul(
                            psO[i],
                            lhsT=C_e[:, cc, ts(i, P)],
                            rhs=eo_all[:, e, cc, :],
                            start=(e == 0 and cc == 0),
                            stop=(e == E - 1 and cc == CC - 1),
                        )
            for i in range(TC // 2):
                o_sb = ostage.tile([P, D], F32, tag="osb")
                nc.scalar.copy(o_sb, psO[i])
                nc.sync.dma_start(out_t[half * (TC // 2) + i], o_sb)

    ctx.close()


_PROGRAM_CACHE = {}


def _get_program():
    if "nc" not in _PROGRAM_CACHE:
        _PROGRAM_CACHE["nc"] = build_program()
    return _PROGRAM_CACHE["nc"]


def _prep_weights(w1, w2):
    # w1s[e, g, p, hh, dc, hp] = w1[e, dc*128+p, (g*8+hh)*128+hp]
    w1s = np.ascontiguousarray(
        w1.reshape(E, DC, P, 2, HC // 2, P).transpose(0, 3, 2, 4, 1, 5)
    ).astype(BF16_NP)
    # w2s[e, g, p, hh, d] = w2[e, (g*8+hh)*128+p, d]
    w2s = np.ascontiguousarray(
        w2.reshape(E, 2, HC // 2, P, D).transpose(0, 1, 3, 2, 4)
    ).astype(BF16_NP)
    return w1s, w2s


def _run(x, w_gating, w1, w2, trace=False, **kwargs):
    from concourse.bass_utils import run_bass_kernel_spmd

    nc = _get_program()
    x = np.asarray(x, dtype=np.float32)
    wg = np.ascontiguousarray(np.asarray(w_gating, dtype=np.float32))
    w1s, w2s = _prep_weights(
        np.asarray(w1, dtype=np.float32), np.asarray(w2, dtype=np.float32)
    )
    in_maps = [
        {
            "x": np.ascontiguousarray(x[b]),
            "wg": wg,
            "w1s": w1s,
            "w2s": w2s,
        }
        for b in range(NCORES)
    ]
    res = run_bass_kernel_spmd(nc, in_maps, list(range(NCORES)), trace=trace, **kwargs)
    out = np.stack([res.results[b]["out"] for b in range(NCORES)], axis=0)
    lps = np.array(
        [np.float64(res.results[b]["lossp"].reshape(())) for b in range(NCORES)]
    )
    loss = np.float32(lps.sum() * E * LOSS_COEF / (float(N) * N * B))
    return (out, loss), res


def kernel(x, w_gating, w1, w2):
    (out, loss), _ = _run(x, w_gating, w1, w2, trace=False)
    return out, loss


if __name__ == "__main__":
    nc = _get_program()
    print("program built and compiled OK")


# revision 21
# speedup vs baseline: 1.2565x; 1.0082x over previous
"""MoE (threshold top-k routing, eval capacity) Trainium2 Bass kernel.

Strategy: data-parallel over the batch dim b (B=8 -> one batch element per
NeuronCore), full expert set computed locally on each core (no collectives).

Per-core program (N=2048 tokens, D=512, E=16, HID=2048, CAP=256):
  1. Gating logits via fp32 PE matmul (precision-critical: routing threshold
     margins are ~3e-5, so the gate path stays fp32 end-to-end).
  2. Softmax on ScalarE/VectorE; threshold top-k WITHOUT sorting via pairwise
     prob comparisons (expert e selected iff sum of probs strictly greater --
     ties broken by index -- is < 0.8).  Bit-matches jnp argsort semantics.
  3. Capacity: exclusive cumsum over tokens via the DVE scan instruction on an
     expert-major [16, 2048] layout; summed-slot quirk (pos_tok = sum over
     experts) reproduced exactly.
  4. Dispatch/combine as one-hot matmuls (handles duplicate-slot collisions by
     summation, exactly like the reference einsum).
  5. Expert FFN (gelu) with bf16 weights x float32r activations, fp32 PSUM.
"""

import sys

import numpy as np

sys.path.insert(0, "/opt/trn_rl_repo")

import ml_dtypes  # noqa: E402

import concourse.bass as bass  # noqa: E402
import concourse.mybir as mybir  # noqa: E402
import concourse.tile as tile  # noqa: E402
from concourse import bacc  # noqa: E402
from concourse.bass import ds, ts  # noqa: E402
from concourse.masks import make_identity  # noqa: E402

F32 = mybir.dt.float32
F32R = mybir.dt.float32r
BF16 = mybir.dt.bfloat16
I32 = mybir.dt.int32
AX = mybir.AxisListType
OP = mybir.AluOpType
AF = mybir.ActivationFunctionType

P = 128
B, N, D, E, HID, CAP = 8, 2048, 512, 16, 2048, 256
TC, DC, HC, CC = N // P, D // P, HID // P, CAP // P
NHALF = N // 2
THRESH = 0.8
LOSS_COEF = 0.01
NCORES = 8

BF16_NP = ml_dtypes.bfloat16


def _r(ap):
    """View an fp32 AP as float32r for 1-cycle/row PE matmuls."""
    return ap.bitcast(F32R)


def build_program():
    nc = bacc.Bacc(
        "TRN2",
        target_bir_lowering=False,
        debug=False,
        enable_asserts=False,
        num_devices=1,
    )

    x_ap = nc.dram_tensor("x", [N, D], F32, kind="ExternalInput").ap()
    wg_ap = nc.dram_tensor("wg", [D, E], F32, kind="ExternalInput").ap()
    # w1s[e, g, p, hh, dc, hp] = w1[e, dc*128+p, (g*8+hh)*128+hp]  (host-prepped)
    w1_ap = nc.dram_tensor("w1s", [E, 2, P, HC // 2, DC, P], BF16, kind="ExternalInput").ap()
    # w2s[e, g, p, hh, d] = w2[e, (g*8+hh)*128+p, d]  (host-prepped)
    w2_ap = nc.dram_tensor("w2s", [E, 2, P, HC // 2, D], BF16, kind="ExternalInput").ap()
    out_ap = nc.dram_tensor("out", [N, D], F32, kind="ExternalOutput").ap()
    lossp_ap = nc.dram_tensor("lossp", [1, 1], F32, kind="ExternalOutput").ap()
    # internal DRAM bounce buffers
    wmd_ap = nc.dram_tensor("wmd", [E, N], F32).ap()

    out_t = out_ap.rearrange("(t p) d -> t p d", p=P)

    with tile.TileContext(nc) as tcx:
        _emit(tcx, x_ap, wg_ap, w1_ap, w2_ap, out_t, lossp_ap, wmd_ap)

    nc.compile()
    return nc


def _emit(tcx, x_ap, wg_ap, w1_ap, w2_ap, out_t, lossp_ap, wmd_ap):
    from contextlib import ExitStack

    nc = tcx.nc
    ctx = ExitStack()
    const = ctx.enter_context(tcx.tile_pool(name="const", bufs=1))
    keep = ctx.enter_context(tcx.tile_pool(name="keep", bufs=1))
    ctxR = ExitStack()
    rp = ctxR.enter_context(tcx.tile_pool(name="routing", bufs=1))
    pw = ctxR.enter_context(tcx.tile_pool(name="pairwise", bufs=2))
    psT = ctxR.enter_context(tcx.tile_pool(name="psT", bufs=6, space="PSUM"))

    # ---------------- constants ----------------
    ident = const.tile([P, P], F32, name="ident")
    make_identity(nc, ident)
    ones16 = const.tile([E, 1], F32, name="ones16")
    nc.vector.memset(ones16, 1.0)
    iota_ci = const.tile([P, CAP], I32, name="iota_ci")
    nc.gpsimd.iota(iota_ci, pattern=[[1, CAP]], base=0, channel_multiplier=0)
    iota_c = const.tile([P, CAP], F32, name="iota_c")
    nc.vector.tensor_copy(iota_c, iota_ci)
    iota_pi = const.tile([P, CC], I32, name="iota_pi")
    nc.gpsimd.iota(iota_pi, pattern=[[P, CC]], base=0, channel_multiplier=1)
    iota_p = const.tile([P, CC], F32, name="iota_p")
    nc.vector.tensor_copy(iota_p, iota_pi)
    # tri[p, ep, e] = 1.0 where e > ep else 0.0 (tie-break mask)
    tri = const.tile([P, E, E], F32, name="tri")
    nc.gpsimd.memset(tri, 1.0)
    nc.gpsimd.affine_select(
        out=tri, in_=tri, pattern=[[-1, E], [1, E]], base=0,
        channel_multiplier=0, compare_op=OP.is_gt, fill=0.0,
    )

    # ---------------- load x, wg ----------------
    x_sb = rp.tile([P, TC, D], F32, name="x_sb")
    nc.sync.dma_start(x_sb, x_ap.rearrange("(t p) d -> p t d", p=P))
    wg_sb = rp.tile([P, DC, E], F32, name="wg_sb")
    nc.sync.dma_start(wg_sb, wg_ap.rearrange("(c p) e -> p c e", p=P))

    # ---------------- transpose x -> xT [d-part, dc, t] ----------------
    with tcx.tile_pool(name="xt", bufs=1) as xtp:
        xT = xtp.tile([P, DC, N], F32, name="xT")
        for t in range(TC):
            for dc in range(DC):
                ps = psT.tile([P, P], F32, tag="ps")
                nc.tensor.transpose(ps, x_sb[:, t, ds(dc * P, P)], ident)
                nc.scalar.copy(xT[:, dc, ds(t * P, P)], ps)

        # ---------------- gating logits [t-part, tc, e] (full fp32) ------
        lg = rp.tile([P, TC, E], F32, name="lg")
        for t in range(TC):
            ps = psT.tile([P, E], F32, tag="ps")
            for dc in range(DC):
                nc.tensor.matmul(
                    ps, lhsT=xT[:, dc, ts(t, P)], rhs=wg_sb[:, dc, :],
                    start=(dc == 0), stop=(dc == DC - 1),
                )
            nc.scalar.copy(lg[:, t, :], ps)

    # persistent f32r copy of x for the dispatch matmuls (after xT freed)
    ctxX = ExitStack()
    xrp = ctxX.enter_context(tcx.tile_pool(name="xrp", bufs=1, side="right"))
    x_r = xrp.tile([P, TC, D], BF16, name="x_r")
    nc.vector.tensor_copy(x_r, x_sb)

    # ---------------- softmax (fp32) ----------------
    rmax = rp.tile([P, TC], F32, name="rmax")
    nc.vector.reduce_max(rmax, lg, axis=AX.X)
    sh = rp.tile([P, TC, E], F32, name="sh")
    nc.vector.tensor_sub(sh, lg, rmax[:, :, None].to_broadcast([P, TC, E]))
    u = rp.tile([P, TC, E], F32, name="u")
    nc.scalar.activation(u, sh, AF.Exp)
    usum = rp.tile([P, TC], F32, name="usum")
    nc.vector.reduce_sum(usum, u, axis=AX.X)
    rin = rp.tile([P, TC], F32, name="rin")
    nc.vector.reciprocal(rin, usum)
    p_sb = rp.tile([P, TC, E], F32, name="p_sb")
    nc.vector.tensor_mul(p_sb, u, rin[:, :, None].to_broadcast([P, TC, E]))

    # ------------- threshold top-k via pairwise comparisons -------------
    # S[t, e] = sum_{e'} p[t,e'] * [ p_e' > p_e  or (p_e' == p_e and e' < e) ]
    # mask[t, e] = S[t, e] < THRESH      (== reference argsort/cumsum mask)
    S = rp.tile([P, TC, E], F32, name="S")
    nc.vector.memset(S, 0.0)
    for ep in range(E):
        eng = nc.vector
        acc = S
        sfx = ""
        pb = p_sb[:, :, ep : ep + 1].to_broadcast([P, TC, E])
        gt = pw.tile([P, TC, E], F32, tag="gt" + sfx)
        eng.tensor_tensor(gt, pb, p_sb, OP.is_gt)
        eq = pw.tile([P, TC, E], F32, tag="eq" + sfx)
        eng.tensor_tensor(eq, pb, p_sb, OP.is_equal)
        # ties only count for columns e > ep
        m = pw.tile([P, TC, E], F32, tag="m" + sfx)
        eng.tensor_mul(m, eq, tri[:, ep : ep + 1, :].to_broadcast([P, TC, E]))
        eng.tensor_add(m, m, gt)
        t2 = pw.tile([P, TC, E], F32, tag="t2" + sfx)
        eng.tensor_mul(t2, m, pb)
        eng.tensor_add(acc, acc, t2)

    mask = rp.tile([P, TC, E], F32, name="mask")
    nc.vector.tensor_single_scalar(mask, S, THRESH, OP.is_lt)
    selp = rp.tile([P, TC, E], F32, name="selp")
    nc.vector.tensor_mul(selp, p_sb, mask)
    wsum = rp.tile([P, TC], F32, name="wsum")
    nc.vector.reduce_sum(wsum, selp, axis=AX.X)
    winv = rp.tile([P, TC], F32, name="winv")
    nc.vector.reciprocal(winv, wsum)
    wts = rp.tile([P, TC, E], F32, name="wts")
    nc.vector.tensor_mul(wts, selp, winv[:, :, None].to_broadcast([P, TC, E]))

    # ------------- transpose mask/wts/p to expert-major [E, N] -------------
    mask_T = rp.tile([E, N], F32, name="mask_T")
    wts_T = rp.tile([E, N], F32, name="wts_T")
    p_T = rp.tile([E, N], F32, name="p_T")
    for t in range(TC):
        for src, dst in ((mask, mask_T), (wts, wts_T), (p_sb, p_T)):
            ps = psT.tile([E, P], F32, tag="ps")
            nc.tensor.transpose(ps, src[:, t, :], ident)
            nc.scalar.copy(dst[:, ds(t * P, P)], ps)

    # ------------- capacity (exclusive cumsum over tokens) -------------
    cum = rp.tile([E, N], F32, name="cum")
    nc.vector.tensor_tensor_scan(cum, mask_T, mask_T, 0.0, OP.add, OP.bypass)
    pos = rp.tile([E, N], F32, name="pos")
    nc.vector.tensor_sub(pos, cum, mask_T)
    mask2_T = rp.tile([E, N], F32, name="mask2_T")
    nc.vector.scalar_tensor_tensor(
        mask2_T, in0=pos, scalar=float(CAP), in1=mask_T, op0=OP.is_lt, op1=OP.mult
    )
    pos2 = rp.tile([E, N], F32, name="pos2")
    nc.vector.tensor_mul(pos2, pos, mask2_T)
    wm_T = rp.tile([E, N], F32, name="wm_T")
    nc.vector.tensor_mul(wm_T, wts_T, mask2_T)
    nc.sync.dma_start(wmd_ap, wm_T)

    # ------------- aux loss partial: sum_e mean_t(p) * mean_t(mask2) -----
    proxs = rp.tile([E, 1], F32, name="proxs")
    nc.vector.reduce_sum(proxs, p_T, axis=AX.X)
    denss = rp.tile([E, 1], F32, name="denss")
    nc.vector.reduce_sum(denss, mask2_T, axis=AX.X)
    prod = rp.tile([E, 1], F32, name="prod")
    nc.vector.tensor_mul(prod, proxs, denss)
    psl = psT.tile([1, 1], F32, tag="ps")
    nc.tensor.matmul(psl, lhsT=prod, rhs=ones16, start=True, stop=True)
    lp_sb = rp.tile([1, 1], F32, name="lp_sb")
    nc.scalar.copy(lp_sb, psl)
    nc.sync.dma_start(lossp_ap, lp_sb)

    # ------------- summed slot index s(t) = sum_e pos2  -------------
    # expert-major broadcast copy: s_bcast[c-part, t]
    s_row = rp.tile([1, N], F32, name="s_row")
    for q in range(N // 512):
        ps = psT.tile([1, 512], F32, tag="ps")
        nc.tensor.matmul(
            ps, lhsT=ones16, rhs=pos2[:, ds(q * 512, 512)], start=True, stop=True
        )
        nc.scalar.copy(s_row[:, ds(q * 512, 512)], ps)
    ones1 = const.tile([1, P], F32, name="ones1")
    nc.vector.memset(ones1, 1.0)
    s_bcast = rp.tile([P, N], F32, name="s_bcast")
    for q in range(N // 512):
        ps = psT.tile([P, 512], F32, tag="ps")
        nc.tensor.matmul(
            ps, lhsT=ones1, rhs=s_row[:, ds(q * 512, 512)], start=True, stop=True
        )
        nc.scalar.copy(s_bcast[:, ds(q * 512, 512)], ps)

    # token-major mask2 and s: transpose back
    mask2_tok = keep.tile([P, TC, E], F32, name="mask2_tok")
    pos2_tok = rp.tile([P, TC, E], F32, name="pos2_tok")
    for t in range(TC):
        for src, dst in ((mask2_T, mask2_tok), (pos2, pos2_tok)):
            ps = psT.tile([P, E], F32, tag="ps")
            nc.tensor.transpose(ps, src[:, ds(t * P, P)], ident[:E, :E])
            nc.scalar.copy(dst[:, t, :], ps)
    s_tok = rp.tile([P, TC], F32, name="s_tok")
    nc.vector.reduce_sum(s_tok, pos2_tok, axis=AX.X)

    # ------------- one-hot slot structures -------------
    oh_tok = keep.tile([P, TC, CAP], BF16, name="oh_tok")  # [t, c] = (s(t)==c)
    nc.vector.tensor_tensor(
        oh_tok,
        s_tok[:, :, None].to_broadcast([P, TC, CAP]),
        iota_c[:, None, :].to_broadcast([P, TC, CAP]),
        OP.is_equal,
    )
    oh_T = keep.tile([P, CC, N], BF16, name="oh_T")  # [c, t] = (s(t)==c)
    for cc in range(CC):
        nc.vector.tensor_tensor(
            oh_T[:, cc, :],
            s_bcast,
            iota_p[:, cc : cc + 1].to_broadcast([P, N]),
            OP.is_equal,
        )

    ctxR.close()

    eoallp = ctx.enter_context(tcx.tile_pool(name="eoall", bufs=1))
    eo_all = eoallp.tile([P, E, CC, D], F32R, name="eo_all")

    # ================= Phase A: dispatch + expert FFN =================
    with (
        tcx.tile_pool(name="mpool", bufs=2) as mpool,
        tcx.tile_pool(name="w1pool", bufs=3) as w1pool,
        tcx.tile_pool(name="w2pool", bufs=3) as w2pool,
        tcx.tile_pool(name="eipool", bufs=3) as eipool,
        tcx.tile_pool(name="hpool", bufs=2) as hpool,
        tcx.tile_pool(name="psA", bufs=1, space="PSUM") as psA,
        tcx.tile_pool(name="psH", bufs=2, space="PSUM") as psH,
    ):
        for e in range(E):
            # M_e[t, c] = oh[t, c] * mask2[t, e]   (bf16, exact 0/1 weights)
            M_e = mpool.tile([P, TC, CAP], BF16, tag="M")
            for t in range(TC):
                nc.vector.tensor_scalar_mul(
                    M_e[:, t, :], oh_tok[:, t, :], mask2_tok[:, t, e : e + 1]
                )
            # EI_T[d, c] = sum_t x[t, d] * M_e[t, c]
            ei = eipool.tile([P, DC, CAP], BF16, tag="ei")
            for dc in range(DC):
                pse = psA.tile([P, CAP], F32, tag=f"ei{dc}")
                for t in range(TC):
                    nc.tensor.matmul(
                        pse,
                        lhsT=x_r[:, t, ds(dc * P, P)],
                        rhs=M_e[:, t, :],
                        start=(t == 0),
                        stop=(t == TC - 1),
                    )
                nc.scalar.copy(ei[:, dc, :], pse)
            # h[hid, c] = gelu(sum_d w1[d, hid] * EI_T[d, c])
            h = hpool.tile([P, HC, CAP], BF16, tag="h")
            for g in range(2):
                w1t = w1pool.tile([P, HC // 2, DC, P], BF16, tag="w1")
                nc.sync.dma_start(w1t, w1_ap[e, g])
                for hh in range(HC // 2):
                    psh = psH.tile([P, CAP], F32, tag="h")
                    for dc in range(DC):
                        nc.tensor.matmul(
                            psh,
                            lhsT=w1t[:, hh, dc, :],
                            rhs=ei[:, dc, :],
                            start=(dc == 0),
                            stop=(dc == DC - 1),
                        )
                    nc.scalar.activation(h[:, g * (HC // 2) + hh, :], psh, AF.Gelu)
            # EO[c, d] = sum_hid h[hid, c] * w2[hid, d]
            pso = [
                psA.tile([P, D], F32, tag=f"eo{cc}", name=f"pso{cc}")
                for cc in range(CC)
            ]
            for g in range(2):
                w2t = w2pool.tile([P, HC // 2, D], BF16, tag="w2")
                nc.sync.dma_start(w2t, w2_ap[e, g])
                for hh in range(HC // 2):
                    hc = g * (HC // 2) + hh
                    for cc in range(CC):
                        nc.tensor.matmul(
                            pso[cc],
                            lhsT=h[:, hc, ds(cc * P, P)],
                            rhs=w2t[:, hh, :],
                            start=(hc == 0),
                            stop=(hc == HC - 1),
                        )
            for cc in range(CC):
                nc.scalar.copy(eo_all[:, e, cc, :], pso[cc])

    ctxX.close()

    # ================= Phase B: combine =================
    with (
        tcx.tile_pool(name="cpool", bufs=3) as cpool,
        tcx.tile_pool(name="wmpool", bufs=4) as wmpool,
        tcx.tile_pool(name="ostage", bufs=3) as ostage,
        tcx.tile_pool(name="psB", bufs=1, space="PSUM") as psB,
    ):
        for half in range(2):
            psO = [
                psB.tile([P, D], F32, tag=f"o{i}", name=f"psO{i}")
                for i in range(TC // 2)
            ]
            for e in range(E):
                wmb = wmpool.tile([P, NHALF], F32, tag="wmb")
                src = wmd_ap[e, ds(half * NHALF, NHALF)]
                src_b = bass.AP(
                    tensor=src.tensor, offset=src.offset, ap=[[0, P], *src.ap]
                )
                nc.gpsimd.dma_start(wmb, src_b)
                C_e = cpool.tile([P, CC, NHALF], F32R, tag="C")
                for cc in range(CC):
                    nc.vector.tensor_mul(
                        C_e[:, cc, :], oh_T[:, cc, ds(half * NHALF, NHALF)], wmb
                    )
                for i in range(TC // 2):
                    for cc in range(CC):
                        nc.tensor.matmul(
                            psO[i],
                            lhsT=C_e[:, cc, ts(i, P)],
                            rhs=eo_all[:, e, cc, :],
                            start=(e == 0 and cc == 0),
                            stop=(e == E - 1 and cc == CC - 1),
                        )
            for i in range(TC // 2):
                o_sb = ostage.tile([P, D], F32, tag="osb")
                nc.scalar.copy(o_sb, psO[i])
                nc.sync.dma_start(out_t[half * (TC // 2) + i], o_sb)

    ctx.close()


_PROGRAM_CACHE = {}


def _get_program():
    if "nc" not in _PROGRAM_CACHE:
        _PROGRAM_CACHE["nc"] = build_program()
    return _PROGRAM_CACHE["nc"]


def _prep_weights(w1, w2):
    # w1s[e, g, p, hh, dc, hp] = w1[e, dc*128+p, (g*8+hh)*128+hp]
    w1s = np.ascontiguousarray(
        w1.reshape(E, DC, P, 2, HC // 2, P).transpose(0, 3, 2, 4, 1, 5)
    ).astype(BF16_NP)
    # w2s[e, g, p, hh, d] = w2[e, (g*8+hh)*128+p, d]
    w2s = np.ascontiguousarray(
        w2.reshape(E, 2, HC // 2, P, D).transpose(0, 1, 3, 2, 4)
    ).astype(BF16_NP)
    return w1s, w2s


def _run(x, w_gating, w1, w2, trace=False, **kwargs):
    from concourse.bass_utils import run_bass_kernel_spmd

    nc = _get_program()
    x = np.asarray(x, dtype=np.float32)
    wg = np.ascontiguousarray(np.asarray(w_gating, dtype=np.float32))
    w1s, w2s = _prep_weights(
        np.asarray(w1, dtype=np.float32), np.asarray(w2, dtype=np.float32)
    )
    in_maps = [
        {
            "x": np.ascontiguousarray(x[b]),
            "wg": wg,
            "w1s": w1s,
            "w2s": w2s,
        }
        for b in range(NCORES)
    ]
    res = run_bass_kernel_spmd(nc, in_maps, list(range(NCORES)), trace=trace, **kwargs)
    out = np.stack([res.results[b]["out"] for b in range(NCORES)], axis=0)
    lps = np.array(
        [np.float64(res.results[b]["lossp"].reshape(())) for b in range(NCORES)]
    )
    loss = np.float32(lps.sum() * E * LOSS_COEF / (float(N) * N * B))
    return (out, loss), res


def kernel(x, w_gating, w1, w2):
    (out, loss), _ = _run(x, w_gating, w1, w2, trace=False)
    return out, loss


if __name__ == "__main__":
    nc = _get_program()
    print("program built and compiled OK")


# revision 22
# speedup vs baseline: 1.2652x; 1.0069x over previous
"""MoE (threshold top-k routing, eval capacity) Trainium2 Bass kernel.

Strategy: data-parallel over the batch dim b (B=8 -> one batch element per
NeuronCore), full expert set computed locally on each core (no collectives).

Per-core program (N=2048 tokens, D=512, E=16, HID=2048, CAP=256):
  1. Gating logits via fp32 PE matmul (precision-critical: routing threshold
     margins are ~3e-5, so the gate path stays fp32 end-to-end).
  2. Softmax on ScalarE/VectorE; threshold top-k WITHOUT sorting via pairwise
     prob comparisons (expert e selected iff sum of probs strictly greater --
     ties broken by index -- is < 0.8).  Bit-matches jnp argsort semantics.
  3. Capacity: exclusive cumsum over tokens via the DVE scan instruction on an
     expert-major [16, 2048] layout; summed-slot quirk (pos_tok = sum over
     experts) reproduced exactly.
  4. Dispatch/combine as one-hot matmuls (handles duplicate-slot collisions by
     summation, exactly like the reference einsum).
  5. Expert FFN (gelu) with bf16 weights x float32r activations, fp32 PSUM.
"""

import sys

import numpy as np

sys.path.insert(0, "/opt/trn_rl_repo")

import ml_dtypes  # noqa: E402

import concourse.bass as bass  # noqa: E402
import concourse.mybir as mybir  # noqa: E402
import concourse.tile as tile  # noqa: E402
from concourse import bacc  # noqa: E402
from concourse.bass import ds, ts  # noqa: E402
from concourse.masks import make_identity  # noqa: E402

F32 = mybir.dt.float32
F32R = mybir.dt.float32r
BF16 = mybir.dt.bfloat16
I32 = mybir.dt.int32
AX = mybir.AxisListType
OP = mybir.AluOpType
AF = mybir.ActivationFunctionType

P = 128
B, N, D, E, HID, CAP = 8, 2048, 512, 16, 2048, 256
TC, DC, HC, CC = N // P, D // P, HID // P, CAP // P
NHALF = N // 2
THRESH = 0.8
LOSS_COEF = 0.01
NCORES = 8

BF16_NP = ml_dtypes.bfloat16


def _r(ap):
    """View an fp32 AP as float32r for 1-cycle/row PE matmuls."""
    return ap.bitcast(F32R)


def build_program():
    nc = bacc.Bacc(
        "TRN2",
        target_bir_lowering=False,
        debug=False,
        enable_asserts=False,
        num_devices=1,
    )

    x_ap = nc.dram_tensor("x", [N, D], F32, kind="ExternalInput").ap()
    wg_ap = nc.dram_tensor("wg", [D, E], F32, kind="ExternalInput").ap()
    # w1s[e, g, p, hh, dc, hp] = w1[e, dc*128+p, (g*8+hh)*128+hp]  (host-prepped)
    w1_ap = nc.dram_tensor("w1s", [E, 2, P, HC // 2, DC, P], BF16, kind="ExternalInput").ap()
    # w2s[e, g, p, hh, d] = w2[e, (g*8+hh)*128+p, d]  (host-prepped)
    w2_ap = nc.dram_tensor("w2s", [E, 2, P, HC // 2, D], BF16, kind="ExternalInput").ap()
    out_ap = nc.dram_tensor("out", [N, D], F32, kind="ExternalOutput").ap()
    lossp_ap = nc.dram_tensor("lossp", [1, 1], F32, kind="ExternalOutput").ap()
    # internal DRAM bounce buffers
    wmd_ap = nc.dram_tensor("wmd", [E, N], BF16).ap()

    out_t = out_ap.rearrange("(t p) d -> t p d", p=P)

    with tile.TileContext(nc) as tcx:
        _emit(tcx, x_ap, wg_ap, w1_ap, w2_ap, out_t, lossp_ap, wmd_ap)

    nc.compile()
    return nc


def _emit(tcx, x_ap, wg_ap, w1_ap, w2_ap, out_t, lossp_ap, wmd_ap):
    from contextlib import ExitStack

    nc = tcx.nc
    ctx = ExitStack()
    const = ctx.enter_context(tcx.tile_pool(name="const", bufs=1))
    keep = ctx.enter_context(tcx.tile_pool(name="keep", bufs=1))
    ctxR = ExitStack()
    rp = ctxR.enter_context(tcx.tile_pool(name="routing", bufs=1))
    pw = ctxR.enter_context(tcx.tile_pool(name="pairwise", bufs=2))
    psT = ctxR.enter_context(tcx.tile_pool(name="psT", bufs=6, space="PSUM"))

    # ---------------- constants ----------------
    ident = const.tile([P, P], F32, name="ident")
    make_identity(nc, ident)
    ones16 = const.tile([E, 1], F32, name="ones16")
    nc.vector.memset(ones16, 1.0)
    iota_ci = const.tile([P, CAP], I32, name="iota_ci")
    nc.gpsimd.iota(iota_ci, pattern=[[1, CAP]], base=0, channel_multiplier=0)
    iota_c = const.tile([P, CAP], F32, name="iota_c")
    nc.vector.tensor_copy(iota_c, iota_ci)
    iota_pi = const.tile([P, CC], I32, name="iota_pi")
    nc.gpsimd.iota(iota_pi, pattern=[[P, CC]], base=0, channel_multiplier=1)
    iota_p = const.tile([P, CC], F32, name="iota_p")
    nc.vector.tensor_copy(iota_p, iota_pi)
    # tri[p, ep, e] = 1.0 where e > ep else 0.0 (tie-break mask)
    tri = const.tile([P, E, E], F32, name="tri")
    nc.gpsimd.memset(tri, 1.0)
    nc.gpsimd.affine_select(
        out=tri, in_=tri, pattern=[[-1, E], [1, E]], base=0,
        channel_multiplier=0, compare_op=OP.is_gt, fill=0.0,
    )

    # ---------------- load x, wg ----------------
    x_sb = rp.tile([P, TC, D], F32, name="x_sb")
    nc.sync.dma_start(x_sb, x_ap.rearrange("(t p) d -> p t d", p=P))
    wg_sb = rp.tile([P, DC, E], F32, name="wg_sb")
    nc.sync.dma_start(wg_sb, wg_ap.rearrange("(c p) e -> p c e", p=P))

    # ---------------- transpose x -> xT [d-part, dc, t] ----------------
    with tcx.tile_pool(name="xt", bufs=1) as xtp:
        xT = xtp.tile([P, DC, N], F32, name="xT")
        for t in range(TC):
            for dc in range(DC):
                ps = psT.tile([P, P], F32, tag="ps")
                nc.tensor.transpose(ps, x_sb[:, t, ds(dc * P, P)], ident)
                nc.scalar.copy(xT[:, dc, ds(t * P, P)], ps)

        # ---------------- gating logits [t-part, tc, e] (full fp32) ------
        lg = rp.tile([P, TC, E], F32, name="lg")
        for t in range(TC):
            ps = psT.tile([P, E], F32, tag="ps")
            for dc in range(DC):
                nc.tensor.matmul(
                    ps, lhsT=xT[:, dc, ts(t, P)], rhs=wg_sb[:, dc, :],
                    start=(dc == 0), stop=(dc == DC - 1),
                )
            nc.scalar.copy(lg[:, t, :], ps)

    # persistent f32r copy of x for the dispatch matmuls (after xT freed)
    ctxX = ExitStack()
    xrp = ctxX.enter_context(tcx.tile_pool(name="xrp", bufs=1, side="right"))
    x_r = xrp.tile([P, TC, D], BF16, name="x_r")
    nc.vector.tensor_copy(x_r, x_sb)

    # ---------------- softmax (fp32) ----------------
    rmax = rp.tile([P, TC], F32, name="rmax")
    nc.vector.reduce_max(rmax, lg, axis=AX.X)
    sh = rp.tile([P, TC, E], F32, name="sh")
    nc.vector.tensor_sub(sh, lg, rmax[:, :, None].to_broadcast([P, TC, E]))
    u = rp.tile([P, TC, E], F32, name="u")
    nc.scalar.activation(u, sh, AF.Exp)
    usum = rp.tile([P, TC], F32, name="usum")
    nc.vector.reduce_sum(usum, u, axis=AX.X)
    rin = rp.tile([P, TC], F32, name="rin")
    nc.vector.reciprocal(rin, usum)
    p_sb = rp.tile([P, TC, E], F32, name="p_sb")
    nc.vector.tensor_mul(p_sb, u, rin[:, :, None].to_broadcast([P, TC, E]))

    # ------------- threshold top-k via pairwise comparisons -------------
    # S[t, e] = sum_{e'} p[t,e'] * [ p_e' > p_e  or (p_e' == p_e and e' < e) ]
    # mask[t, e] = S[t, e] < THRESH      (== reference argsort/cumsum mask)
    S = rp.tile([P, TC, E], F32, name="S")
    nc.vector.memset(S, 0.0)
    for ep in range(E):
        eng = nc.vector
        acc = S
        sfx = ""
        pb = p_sb[:, :, ep : ep + 1].to_broadcast([P, TC, E])
        gt = pw.tile([P, TC, E], F32, tag="gt" + sfx)
        eng.tensor_tensor(gt, pb, p_sb, OP.is_gt)
        eq = pw.tile([P, TC, E], F32, tag="eq" + sfx)
        eng.tensor_tensor(eq, pb, p_sb, OP.is_equal)
        # ties only count for columns e > ep
        m = pw.tile([P, TC, E], F32, tag="m" + sfx)
        eng.tensor_mul(m, eq, tri[:, ep : ep + 1, :].to_broadcast([P, TC, E]))
        eng.tensor_add(m, m, gt)
        t2 = pw.tile([P, TC, E], F32, tag="t2" + sfx)
        eng.tensor_mul(t2, m, pb)
        eng.tensor_add(acc, acc, t2)

    mask = rp.tile([P, TC, E], F32, name="mask")
    nc.vector.tensor_single_scalar(mask, S, THRESH, OP.is_lt)
    selp = rp.tile([P, TC, E], F32, name="selp")
    nc.vector.tensor_mul(selp, p_sb, mask)
    wsum = rp.tile([P, TC], F32, name="wsum")
    nc.vector.reduce_sum(wsum, selp, axis=AX.X)
    winv = rp.tile([P, TC], F32, name="winv")
    nc.vector.reciprocal(winv, wsum)
    wts = rp.tile([P, TC, E], F32, name="wts")
    nc.vector.tensor_mul(wts, selp, winv[:, :, None].to_broadcast([P, TC, E]))

    # ------------- transpose mask/wts/p to expert-major [E, N] -------------
    mask_T = rp.tile([E, N], F32, name="mask_T")
    wts_T = rp.tile([E, N], F32, name="wts_T")
    p_T = rp.tile([E, N], F32, name="p_T")
    for t in range(TC):
        for src, dst in ((mask, mask_T), (wts, wts_T), (p_sb, p_T)):
            ps = psT.tile([E, P], F32, tag="ps")
            nc.tensor.transpose(ps, src[:, t, :], ident)
            nc.scalar.copy(dst[:, ds(t * P, P)], ps)

    # ------------- capacity (exclusive cumsum over tokens) -------------
    cum = rp.tile([E, N], F32, name="cum")
    nc.vector.tensor_tensor_scan(cum, mask_T, mask_T, 0.0, OP.add, OP.bypass)
    pos = rp.tile([E, N], F32, name="pos")
    nc.vector.tensor_sub(pos, cum, mask_T)
    mask2_T = rp.tile([E, N], F32, name="mask2_T")
    nc.vector.scalar_tensor_tensor(
        mask2_T, in0=pos, scalar=float(CAP), in1=mask_T, op0=OP.is_lt, op1=OP.mult
    )
    pos2 = rp.tile([E, N], F32, name="pos2")
    nc.vector.tensor_mul(pos2, pos, mask2_T)
    wm_T = rp.tile([E, N], F32, name="wm_T")
    nc.vector.tensor_mul(wm_T, wts_T, mask2_T)
    wm_b = rp.tile([E, N], BF16, name="wm_b")
    nc.vector.tensor_copy(wm_b, wm_T)
    nc.sync.dma_start(wmd_ap, wm_b)

    # ------------- aux loss partial: sum_e mean_t(p) * mean_t(mask2) -----
    proxs = rp.tile([E, 1], F32, name="proxs")
    nc.vector.reduce_sum(proxs, p_T, axis=AX.X)
    denss = rp.tile([E, 1], F32, name="denss")
    nc.vector.reduce_sum(denss, mask2_T, axis=AX.X)
    prod = rp.tile([E, 1], F32, name="prod")
    nc.vector.tensor_mul(prod, proxs, denss)
    psl = psT.tile([1, 1], F32, tag="ps")
    nc.tensor.matmul(psl, lhsT=prod, rhs=ones16, start=True, stop=True)
    lp_sb = rp.tile([1, 1], F32, name="lp_sb")
    nc.scalar.copy(lp_sb, psl)
    nc.sync.dma_start(lossp_ap, lp_sb)

    # ------------- summed slot index s(t) = sum_e pos2  -------------
    # expert-major broadcast copy: s_bcast[c-part, t]
    s_row = rp.tile([1, N], F32, name="s_row")
    for q in range(N // 512):
        ps = psT.tile([1, 512], F32, tag="ps")
        nc.tensor.matmul(
            ps, lhsT=ones16, rhs=pos2[:, ds(q * 512, 512)], start=True, stop=True
        )
        nc.scalar.copy(s_row[:, ds(q * 512, 512)], ps)
    ones1 = const.tile([1, P], F32, name="ones1")
    nc.vector.memset(ones1, 1.0)
    s_bcast = rp.tile([P, N], F32, name="s_bcast")
    for q in range(N // 512):
        ps = psT.tile([P, 512], F32, tag="ps")
        nc.tensor.matmul(
            ps, lhsT=ones1, rhs=s_row[:, ds(q * 512, 512)], start=True, stop=True
        )
        nc.scalar.copy(s_bcast[:, ds(q * 512, 512)], ps)

    # token-major mask2 and s: transpose back
    mask2_tok = keep.tile([P, TC, E], F32, name="mask2_tok")
    pos2_tok = rp.tile([P, TC, E], F32, name="pos2_tok")
    for t in range(TC):
        for src, dst in ((mask2_T, mask2_tok), (pos2, pos2_tok)):
            ps = psT.tile([P, E], F32, tag="ps")
            nc.tensor.transpose(ps, src[:, ds(t * P, P)], ident[:E, :E])
            nc.scalar.copy(dst[:, t, :], ps)
    s_tok = rp.tile([P, TC], F32, name="s_tok")
    nc.vector.reduce_sum(s_tok, pos2_tok, axis=AX.X)

    # ------------- one-hot slot structures -------------
    oh_tok = keep.tile([P, TC, CAP], BF16, name="oh_tok")  # [t, c] = (s(t)==c)
    nc.vector.tensor_tensor(
        oh_tok,
        s_tok[:, :, None].to_broadcast([P, TC, CAP]),
        iota_c[:, None, :].to_broadcast([P, TC, CAP]),
        OP.is_equal,
    )
    oh_T = keep.tile([P, CC, N], BF16, name="oh_T")  # [c, t] = (s(t)==c)
    for cc in range(CC):
        nc.vector.tensor_tensor(
            oh_T[:, cc, :],
            s_bcast,
            iota_p[:, cc : cc + 1].to_broadcast([P, N]),
            OP.is_equal,
        )

    ctxR.close()

    eoallp = ctx.enter_context(tcx.tile_pool(name="eoall", bufs=1))
    eo_all = eoallp.tile([P, E, CC, D], BF16, name="eo_all")

    # ================= Phase A: dispatch + expert FFN =================
    with (
        tcx.tile_pool(name="mpool", bufs=2) as mpool,
        tcx.tile_pool(name="w1pool", bufs=3) as w1pool,
        tcx.tile_pool(name="w2pool", bufs=3) as w2pool,
        tcx.tile_pool(name="eipool", bufs=3) as eipool,
        tcx.tile_pool(name="hpool", bufs=2) as hpool,
        tcx.tile_pool(name="psA", bufs=1, space="PSUM") as psA,
        tcx.tile_pool(name="psH", bufs=2, space="PSUM") as psH,
    ):
        for e in range(E):
            # M_e[t, c] = oh[t, c] * mask2[t, e]   (bf16, exact 0/1 weights)
            M_e = mpool.tile([P, TC, CAP], BF16, tag="M")
            for t in range(TC):
                nc.vector.tensor_scalar_mul(
                    M_e[:, t, :], oh_tok[:, t, :], mask2_tok[:, t, e : e + 1]
                )
            # EI_T[d, c] = sum_t x[t, d] * M_e[t, c]
            ei = eipool.tile([P, DC, CAP], BF16, tag="ei")
            for dc in range(DC):
                pse = psA.tile([P, CAP], F32, tag=f"ei{dc}")
                for t in range(TC):
                    nc.tensor.matmul(
                        pse,
                        lhsT=x_r[:, t, ds(dc * P, P)],
                        rhs=M_e[:, t, :],
                        start=(t == 0),
                        stop=(t == TC - 1),
                    )
                nc.scalar.copy(ei[:, dc, :], pse)
            # h[hid, c] = gelu(sum_d w1[d, hid] * EI_T[d, c])
            h = hpool.tile([P, HC, CAP], BF16, tag="h")
            for g in range(2):
                w1t = w1pool.tile([P, HC // 2, DC, P], BF16, tag="w1")
                nc.sync.dma_start(w1t, w1_ap[e, g])
                for hh in range(HC // 2):
                    psh = psH.tile([P, CAP], F32, tag="h")
                    for dc in range(DC):
                        nc.tensor.matmul(
                            psh,
                            lhsT=w1t[:, hh, dc, :],
                            rhs=ei[:, dc, :],
                            start=(dc == 0),
                            stop=(dc == DC - 1),
                        )
                    nc.scalar.activation(h[:, g * (HC // 2) + hh, :], psh, AF.Gelu)
            # EO[c, d] = sum_hid h[hid, c] * w2[hid, d]
            pso = [
                psA.tile([P, D], F32, tag=f"eo{cc}", name=f"pso{cc}")
                for cc in range(CC)
            ]
            for g in range(2):
                w2t = w2pool.tile([P, HC // 2, D], BF16, tag="w2")
                nc.sync.dma_start(w2t, w2_ap[e, g])
                for hh in range(HC // 2):
                    hc = g * (HC // 2) + hh
                    for cc in range(CC):
                        nc.tensor.matmul(
                            pso[cc],
                            lhsT=h[:, hc, ds(cc * P, P)],
                            rhs=w2t[:, hh, :],
                            start=(hc == 0),
                            stop=(hc == HC - 1),
                        )
            for cc in range(CC):
                nc.scalar.copy(eo_all[:, e, cc, :], pso[cc])

    ctxX.close()

    # ================= Phase B: combine =================
    with (
        tcx.tile_pool(name="cpool", bufs=3) as cpool,
        tcx.tile_pool(name="wmpool", bufs=4) as wmpool,
        tcx.tile_pool(name="ostage", bufs=3) as ostage,
        tcx.tile_pool(name="psB", bufs=1, space="PSUM") as psB,
    ):
        for half in range(2):
            psO = [
                psB.tile([P, D], F32, tag=f"o{i}", name=f"psO{i}")
                for i in range(TC // 2)
            ]
            for e in range(E):
                wmb = wmpool.tile([P, NHALF], BF16, tag="wmb")
                src = wmd_ap[e, ds(half * NHALF, NHALF)]
                src_b = bass.AP(
                    tensor=src.tensor, offset=src.offset, ap=[[0, P], *src.ap]
                )
                nc.gpsimd.dma_start(wmb, src_b)
                C_e = cpool.tile([P, CC, NHALF], BF16, tag="C")
                for cc in range(CC):
                    nc.vector.tensor_mul(
                        C_e[:, cc, :], oh_T[:, cc, ds(half * NHALF, NHALF)], wmb
                    )
                for i in range(TC // 2):
                    for cc in range(CC):
                        nc.tensor.matmul(
                            psO[i],
                            lhsT=C_e[:, cc, ts(i, P)],
                            rhs=eo_all[:, e, cc, :],
                            start=(e == 0 and cc == 0),
                            stop=(e == E - 1 and cc == CC - 1),
                        )
            for i in range(TC // 2):
                o_sb = ostage.tile([P, D], F32, tag="osb")
                nc.scalar.copy(o_sb, psO[i])
                nc.sync.dma_start(out_t[half * (TC // 2) + i], o_sb)

    ctx.close()


_PROGRAM_CACHE = {}


def _get_program():
    if "nc" not in _PROGRAM_CACHE:
        _PROGRAM_CACHE["nc"] = build_program()
    return _PROGRAM_CACHE["nc"]


def _prep_weights(w1, w2):
    # w1s[e, g, p, hh, dc, hp] = w1[e, dc*128+p, (g*8+hh)*128+hp]
    w1s = np.ascontiguousarray(
        w1.reshape(E, DC, P, 2, HC // 2, P).transpose(0, 3, 2, 4, 1, 5)
    ).astype(BF16_NP)
    # w2s[e, g, p, hh, d] = w2[e, (g*8+hh)*128+p, d]
    w2s = np.ascontiguousarray(
        w2.reshape(E, 2, HC // 2, P, D).transpose(0, 1, 3, 2, 4)
    ).astype(BF16_NP)
    return w1s, w2s


def _run(x, w_gating, w1, w2, trace=False, **kwargs):
    from concourse.bass_utils import run_bass_kernel_spmd

    nc = _get_program()
    x = np.asarray(x, dtype=np.float32)
    wg = np.ascontiguousarray(np.asarray(w_gating, dtype=np.float32))
    w1s, w2s = _prep_weights(
        np.asarray(w1, dtype=np.float32), np.asarray(w2, dtype=np.float32)
    )
    in_maps = [
        {
            "x": np.ascontiguousarray(x[b]),
            "wg": wg,
            "w1s": w1s,
            "w2s": w2s,
        }
        for b in range(NCORES)
    ]
    res = run_bass_kernel_spmd(nc, in_maps, list(range(NCORES)), trace=trace, **kwargs)
    out = np.stack([res.results[b]["out"] for b in range(NCORES)], axis=0)
    lps = np.array(
        [np.float64(res.results[b]["lossp"].reshape(())) for b in range(NCORES)]
    )
    loss = np.float32(lps.sum() * E * LOSS_COEF / (float(N) * N * B))
    return (out, loss), res


def kernel(x, w_gating, w1, w2):
    (out, loss), _ = _run(x, w_gating, w1, w2, trace=False)
    return out, loss


if __name__ == "__main__":
    nc = _get_program()
    print("program built and compiled OK")
